# revision 17
# baseline (speedup 1.0000x reference)
"""Trainium2 Bass kernel for nn_CapsuleLayer (B=64, L=512, D=1024, C=32, O=64).

Strategy: data-parallel over batch across 8 NeuronCores (8 batch elements per
core), processed in 4 pipelined groups of 2 so that projection of group k+1
overlaps routing of group k on the PE. Per core, per batch element:
  u_hatT[co, l] = fc_w.T @ xT (+ fc_b)      PE, fc_w stationary
  u_hat[l, co]  = transpose(u_hatT)          PE transpose
  routing, all on-chip:
    iter 0 is algebraic: c_ij uniform => s0 = (sum_l x) @ W / 32 + 16*bias,
      computed once for all 8 batch elements with a tiny GEMV at kernel start
    s_j  = diag-extract(c_ij.T @ u_hat)      PE cross-matmul, batches packed
                                             into PSUM partitions, bank-major
                                             with a 2-bank rotation
    v_j  = squash(s_j)                       ACT/DVE
    b_ij = u_hat . (v_0+..+v_t)              PE: 16 accumulating matmuls with
                                             block-diag masked weights built
                                             from the accumulated v (W-acc)

Everything except batch sharding + input transpose/cast + x column-sums
happens on device.
"""

import contextlib
import ctypes
import sys
import types

import numpy as np
import ml_dtypes

B, L, D = 64, 512, 1024
C, O = 32, 64
CO = C * O                  # 2048
ITERS = 3
NCORES = 8
BPC = B // NCORES           # 8 batch elements per core
GB = 2                      # batch elements per routing group
NGRP = BPC // GB            # 4
P = 128
KD = D // P                 # 8 contraction chunks
MT = CO // P                # 16 m-tiles of u_hatT (= capsule pairs)
LT = L // P                 # 4 l-chunks
NBANK = CO // 512           # 4 cross-matmul column banks
SP = GB * C                 # s-cross partitions (64)

_BF16 = ml_dtypes.bfloat16

# ---------------------------------------------------------------------------
# NTFF profiling shim (used when tracing is requested by the test harness)
# ---------------------------------------------------------------------------


def _install_ntff_shim():
    if "antenv.axon_hooks" in sys.modules:
        return
    so_path = "/opt/axon/libaxon_pjrt.so"
    hook = None
    try:
        lib = ctypes.CDLL(so_path)
        if hasattr(lib, "axon_start_nrt_profile"):
            lib.axon_start_nrt_profile.argtypes = [
                ctypes.POINTER(ctypes.c_int64),
                ctypes.c_size_t,
            ]
            lib.axon_start_nrt_profile.restype = ctypes.c_int64
            lib.axon_stop_nrt_profile.argtypes = [ctypes.c_char_p]
            lib.axon_stop_nrt_profile.restype = ctypes.c_int64

            @contextlib.contextmanager
            def hook(output_dir, device_ids):
                import jax

                jax.devices()
                if device_ids:
                    ids = (ctypes.c_int64 * len(device_ids))(*device_ids)
                    rc = lib.axon_start_nrt_profile(ids, len(device_ids))
                else:
                    rc = lib.axon_start_nrt_profile(None, 0)
                if rc != 0:
                    raise RuntimeError(f"axon_start_nrt_profile rc={rc}")
                try:
                    yield
                finally:
                    n = lib.axon_stop_nrt_profile(str(output_dir).encode())
                    if n < 0:
                        raise RuntimeError(f"axon_stop_nrt_profile rc={n}")
    except OSError:
        pass
    mod = types.ModuleType("antenv.axon_hooks")
    mod.get_axon_ntff_profile_hook = lambda: hook
    mod.set_axon_ntff_profile_hook = lambda h: None
    sys.modules["antenv.axon_hooks"] = mod

    import concourse.bass_utils as bu

    bu.upload_artifacts = lambda tmpdir: tmpdir


# ---------------------------------------------------------------------------
# Kernel builder
# ---------------------------------------------------------------------------


def build_kernel():
    import concourse.bacc as bacc
    import concourse.tile as tile
    import concourse.mybir as mybir

    f32 = mybir.dt.float32
    bf16 = mybir.dt.bfloat16
    AF = mybir.ActivationFunctionType
    ALU = mybir.AluOpType
    AX = mybir.AxisListType

    nc = bacc.Bacc("TRN2", target_bir_lowering=False, debug=False)

    xt_d = nc.dram_tensor("xt", [BPC, D, L], bf16, kind="ExternalInput")
    w_d = nc.dram_tensor("w", [D, CO], bf16, kind="ExternalInput")
    bias_d = nc.dram_tensor("bias_t", [P, MT], f32, kind="ExternalInput")
    xsumt_d = nc.dram_tensor("xsumt", [D, BPC], bf16, kind="ExternalInput")
    bias16_d = nc.dram_tensor("bias16", [BPC, CO], bf16, kind="ExternalInput")
    ident_d = nc.dram_tensor("ident", [P, P], bf16, kind="ExternalInput")
    identf_d = nc.dram_tensor("identf", [C, C], f32, kind="ExternalInput")
    identf128_d = nc.dram_tensor("identf128", [P, P], f32, kind="ExternalInput")
    m0_d = nc.dram_tensor("m0", [P, MT * C], bf16, kind="ExternalInput")
    maskx_d = nc.dram_tensor("mask_x", [P, C], f32, kind="ExternalInput")
    out_d = nc.dram_tensor("v", [BPC * C, O], f32, kind="ExternalOutput")

    with tile.TileContext(nc) as tc, contextlib.ExitStack() as glb:
        const_pool = glb.enter_context(tc.tile_pool(name="consts", bufs=1))
        w_pool = glb.enter_context(tc.tile_pool(name="w", bufs=KD))
        gscr = glb.enter_context(tc.tile_pool(name="gscr", bufs=1))
        xt_pool = glb.enter_context(tc.tile_pool(name="xt", bufs=2 * KD))
        ut_pool = glb.enter_context(tc.tile_pool(name="ut", bufs=2 * GB * MT))
        u_pool = glb.enter_context(tc.tile_pool(name="u", bufs=2 * GB * LT - 1))
        sm_pool = glb.enter_context(tc.tile_pool(name="sm", bufs=2))
        pp_mm = glb.enter_context(tc.tile_pool(name="ppmm", bufs=3, space="PSUM"))
        ps_s_pool = glb.enter_context(tc.tile_pool(name="pss", bufs=2, space="PSUM"))
        ps_d_pool = glb.enter_context(tc.tile_pool(name="psd", bufs=1, space="PSUM"))
        ps_t_pool = glb.enter_context(tc.tile_pool(name="pst", bufs=1, space="PSUM"))

        # --- constants; g0 xt DMAs issued early so the PE can start ---
        w_sb = []
        for k in range(KD):
            wt = w_pool.tile([P, CO], bf16, tag="w", name=f"w{k}")
            nc.sync.dma_start(wt[:], w_d[k * P:(k + 1) * P, :])
            w_sb.append(wt)
        xt_pre = {}
        for i in range(GB):
            for k in range(KD):
                t = xt_pool.tile([P, L], bf16, tag="xt", name=f"xt_g0_{i}_{k}")
                nc.sync.dma_start(t[:], xt_d[i, k * P:(k + 1) * P, :])
                xt_pre[i, k] = t
        ident = const_pool.tile([P, P], bf16, name="ident")
        nc.sync.dma_start(ident[:], ident_d[:])
        bias_sb = const_pool.tile([P, MT], f32, name="bias_sb")
        nc.sync.dma_start(bias_sb[:], bias_d[:])
        identf = const_pool.tile([C, C], f32, name="identf")
        nc.scalar.dma_start(identf[:], identf_d[:])
        identf128 = const_pool.tile([P, P], f32, name="identf128")
        nc.scalar.dma_start(identf128[:], identf128_d[:])
        m0 = const_pool.tile([P, MT * C], bf16, name="m0")
        nc.scalar.dma_start(m0[:], m0_d[:])
        maskx = const_pool.tile([P, C], f32, name="maskx")
        nc.scalar.dma_start(maskx[:], maskx_d[:])
        eps_sb = const_pool.tile([P, 1], f32, name="eps_sb")
        nc.vector.memset(eps_sb[:], 1e-8)
        # ttv[(j,o), m, b]: partition j*64+o holds v0[b, c=2m+j, o]
        ttv = const_pool.tile([P, MT, BPC], bf16, name="ttv")

        # --- iter-0 GEMV: s0 = xsum @ W / 32 + 16*bias for all 8 elems.
        # Issued into the PE queue between the two g0 projections; the tiny
        # squash chain runs on DVE/ACT under the second projection.
        xs_sb = gscr.tile([P, KD, BPC], bf16, name="xs_sb")
        nc.sync.dma_start(
            xs_sb[:], xsumt_d[:].rearrange("(k p) b -> p k b", p=P))
        bias16_sb = gscr.tile([BPC, CO], bf16, tag="g8a", name="bias16")
        nc.sync.dma_start(bias16_sb[:], bias16_d[:])
        v0_sb = gscr.tile([BPC, CO], bf16, name="v0_sb")

        def gemv_mm():
            s0_sb = gscr.tile([BPC, CO], bf16, name="s0_sb")
            for n in range(NBANK):
                ps0 = pp_mm.tile([BPC, 512], f32, tag="mm", name=f"ps0_{n}")
                for k in range(KD):
                    nc.tensor.matmul(
                        ps0[:], xs_sb[:, k, :],
                        w_sb[k][:, n * 512:(n + 1) * 512],
                        start=(k == 0), stop=(k == KD - 1),
                    )
                nc.vector.scalar_tensor_tensor(
                    s0_sb[:, n * 512:(n + 1) * 512], ps0[:], 1.0 / 32.0,
                    bias16_sb[:, n * 512:(n + 1) * 512], ALU.mult, ALU.add,
                )
            # squash s0 -> v0  (scale varies along free dim (c))
            tmp0 = gscr.tile([BPC, CO], bf16, tag="g8a", name="tmp0")
            nc.vector.tensor_tensor(tmp0[:], s0_sb[:], s0_sb[:], ALU.mult)
            sq0 = gscr.tile([BPC, C], f32, name="sq0")
            nc.vector.tensor_reduce(
                sq0[:], tmp0[:].rearrange("b (c o) -> b c o", c=C),
                AX.X, ALU.add)
            r10 = gscr.tile([BPC, C], f32, name="r10")
            nc.scalar.activation(r10[:], sq0[:], AF.Sqrt, bias=eps_sb[:BPC])
            r20 = gscr.tile([BPC, C], f32, name="r20")
            nc.vector.scalar_tensor_tensor(
                r20[:], sq0[:], 1.0, r10[:], ALU.add, ALU.mult)
            rr0 = gscr.tile([BPC, C], f32, name="rr0")
            nc.vector.reciprocal(rr0[:], r20[:])
            sc0 = gscr.tile([BPC, C], f32, name="sc0")
            nc.vector.tensor_tensor(sc0[:], sq0[:], rr0[:], ALU.mult)
            nc.vector.tensor_tensor(
                v0_sb[:].rearrange("b (c o) -> b c o", c=C),
                s0_sb[:].rearrange("b (c o) -> b c o", c=C),
                sc0[:].unsqueeze(2).broadcast_to((BPC, C, O)),
                ALU.mult,
            )

        def gemv_tt():
            tt_ps = ps_t_pool.tile([P, MT * BPC], bf16, tag="dt",
                                   name="tt_ps")
            for m in range(MT):
                nc.tensor.matmul(
                    tt_ps[:, m * BPC:(m + 1) * BPC],
                    v0_sb[:, m * P:(m + 1) * P],
                    ident[:BPC, :BPC],
                    is_transpose=True,
                    start=(m == 0), stop=(m == MT - 1),
                )
            nc.vector.tensor_copy(
                ttv[:], tt_ps[:].rearrange("p (m b) -> p m b", m=MT))

        for g in range(NGRP):
            bs = [g * GB + i for i in range(GB)]  # absolute batch ids

            # ---------------- projection + transpose phase ----------------
            UT = {}  # (i, m) -> [P, L] bf16, partitions = co chunk m
            U = {}   # (i, lt) -> [P, CO] bf16, partitions = l chunk lt
            for i in range(GB):
                for lt in range(LT):
                    U[i, lt] = u_pool.tile([P, CO], bf16, tag="u",
                                           name=f"u_g{g}_{i}_{lt}")

            for i, b in enumerate(bs):
                if g == 0 and i == 1:
                    gemv_mm()
                xt_sb = {}
                for k in range(KD):
                    if g == 0:
                        xt_sb[k] = xt_pre[i, k]
                        continue
                    t = xt_pool.tile([P, L], bf16, tag="xt",
                                     name=f"xt_g{g}_{i}_{k}")
                    nc.sync.dma_start(t[:], xt_d[b, k * P:(k + 1) * P, :])
                    xt_sb[k] = t

                for m in range(MT):
                    ps = pp_mm.tile([P, 512], f32, tag="mm",
                                    name=f"ps_g{g}_{m}_{i}")
                    for k in range(KD):
                        nc.tensor.matmul(
                            ps[:],
                            w_sb[k][:, m * P:(m + 1) * P],
                            xt_sb[k][:],
                            start=(k == 0),
                            stop=(k == KD - 1),
                        )
                    ut = ut_pool.tile([P, L], bf16, tag="ut",
                                      name=f"ut_g{g}_{i}_{m}")
                    # u_hatT = psum + bias (bias varies per partition=co)
                    nc.scalar.activation(
                        ut[:], ps[:], AF.Identity,
                        bias=bias_sb[:, m:m + 1],
                    )
                    UT[i, m] = ut
                    # un-transpose via the DMA XBAR (keeps the PE free);
                    # alternate dispatch between the two HWDGE queues
                    for lt in range(LT):
                        nc.sync.dma_start(
                            U[i, lt][:, m * P:(m + 1) * P],
                            ut[:, lt * P:(lt + 1) * P],
                            transpose=True,
                        )

            if g == 0:
                gemv_tt()

            # ---------------- routing phase ----------------
            # b_ij after iter t is always u_hat . (v_0+..+v_t): accumulate the
            # block-diag masked weights W (linear in v) instead of b_ij.

            def s_pass_and_squash(it, c_get, g=g, U=U):
                # bank-major with a 2-bank psum rotation; extraction of
                # bank n overlaps the s-matmuls of bank n+1
                s01 = sm_pool.tile([SP, O], f32, tag="s01",
                                   name=f"s01_g{g}_{it}")
                s23 = sm_pool.tile([SP, O], f32, tag="s23",
                                   name=f"s23_g{g}_{it}")
                sps = []
                for n in range(NBANK):
                    ps_s = ps_s_pool.tile([SP, 512], f32, tag="ss",
                                          name=f"pss_g{g}_{it}_{n}")
                    for lt in range(LT):
                        for i in range(GB):
                            nc.tensor.matmul(
                                ps_s[i * C:(i + 1) * C, :],
                                c_get(i, lt),
                                U[i, lt][:, n * 512:(n + 1) * 512],
                                start=(lt == 0),
                                stop=(lt == LT - 1),
                                tile_position=(0, i * C),
                            )
                    tmpb = sm_pool.tile([SP, 512], f32, tag="tmpb",
                                        name=f"tmpb_g{g}_{it}_{n}")
                    nc.vector.tensor_tensor(
                        tmpb[:].rearrange("p (c o) -> p c o", c=8),
                        ps_s[:].rearrange("p (c o) -> p c o", c=8),
                        maskx[:SP, n * 8:(n + 1) * 8].unsqueeze(2)
                            .broadcast_to((SP, 8, O)),
                        ALU.mult,
                    )
                    sp = sm_pool.tile([SP, O], f32, tag=f"spart{n}", bufs=1,
                                      name=f"sp_g{g}_{it}_{n}")
                    nc.vector.tensor_reduce(
                        sp[:],
                        tmpb[:].rearrange("p (c o) -> p o c", c=8),
                        AX.X, ALU.add,
                    )
                    sps.append(sp)
                    if n == 1:
                        nc.vector.tensor_tensor(s01[:], sps[0][:], sps[1][:],
                                                ALU.add)
                    elif n == 3:
                        nc.vector.tensor_tensor(s23[:], sps[2][:], sps[3][:],
                                                ALU.add)
                s_all = sm_pool.tile([SP, O], f32, tag="sall",
                                     name=f"sall_g{g}_{it}")
                nc.vector.tensor_tensor(s_all[:], s01[:], s23[:], ALU.add)

                # squash: v = s * sq/(1+sq)/sqrt(sq+1e-8)
                ssq = sm_pool.tile([SP, O], f32, tag="ssq", bufs=1,
                                   name=f"ssq_g{g}_{it}")
                sq = sm_pool.tile([SP, 1], f32, tag="sq",
                                  name=f"sq_g{g}_{it}")
                nc.scalar.activation(ssq[:], s_all[:], AF.Square,
                                     accum_out=sq[:])
                r1 = sm_pool.tile([SP, 1], f32, tag="r1",
                                  name=f"r1_g{g}_{it}")
                nc.scalar.activation(r1[:], sq[:], AF.Sqrt, bias=eps_sb[:SP])
                r2 = sm_pool.tile([SP, 1], f32, tag="r2",
                                  name=f"r2_g{g}_{it}")
                nc.vector.scalar_tensor_tensor(
                    r2[:], sq[:], 1.0, r1[:], ALU.add, ALU.mult,
                )
                rr = sm_pool.tile([SP, 1], f32, tag="rr",
                                  name=f"rr_g{g}_{it}")
                nc.vector.reciprocal(rr[:], r2[:])
                v_all = sm_pool.tile([SP, O], f32, tag="vall",
                                     name=f"vall_g{g}_{it}")
                # v = (s * sq) * (1 / ((1+sq) sqrt(sq+eps)))
                nc.vector.tensor_scalar(
                    v_all[:], s_all[:], sq[:], rr[:], ALU.mult, ALU.mult,
                )
                return v_all

            c_cur = [None]

            def c_cur_get(i, lt):
                return c_cur[0][:, i * LT + lt, :]

            W_cur = {}
            for it in range(ITERS - 1):
                if it == 0:
                    # W(v0) from the precomputed transposed v0 (ttv)
                    for i in range(GB):
                        W0 = sm_pool.tile([P, MT, C], bf16, tag=f"W{i}",
                                          name=f"W_g{g}_{it}_{i}")
                        nc.vector.tensor_tensor(
                            W0[:O],
                            ttv[:O, :, bs[i]].unsqueeze(2)
                                .broadcast_to((O, MT, C)),
                            m0[:O].rearrange("p (t c) -> p t c", t=MT),
                            ALU.mult,
                        )
                        nc.vector.tensor_tensor(
                            W0[O:],
                            ttv[O:, :, bs[i]].unsqueeze(2)
                                .broadcast_to((O, MT, C)),
                            m0[O:].rearrange("p (t c) -> p t c", t=MT),
                            ALU.mult,
                        )
                        W_cur[i] = W0
                else:
                    v_all = s_pass_and_squash(it, c_cur_get)
                    # vT_all[o, (i,c)] = v_all[(i,c), o]
                    ps_v = ps_t_pool.tile([O, SP], f32, tag="dt",
                                          name=f"psv_g{g}_{it}")
                    nc.tensor.transpose(ps_v[:], v_all[:],
                                        identf128[:SP, :SP])
                    ps_v = ps_v[:]
                    for i in range(GB):
                        Wv = sm_pool.tile([P, MT, C], bf16, tag="Wv",
                                          name=f"Wv_g{g}_{it}_{i}")
                        nc.vector.tensor_tensor(
                            Wv[:O],
                            ps_v[:, i * C:(i + 1) * C:2].unsqueeze(2)
                                .broadcast_to((O, MT, C)),
                            m0[:O].rearrange("p (t c) -> p t c", t=MT),
                            ALU.mult,
                        )
                        nc.vector.tensor_tensor(
                            Wv[O:],
                            ps_v[:, i * C + 1:(i + 1) * C:2].unsqueeze(2)
                                .broadcast_to((O, MT, C)),
                            m0[O:].rearrange("p (t c) -> p t c", t=MT),
                            ALU.mult,
                        )
                        W1 = sm_pool.tile([P, MT, C], bf16, tag=f"W{i}",
                                          name=f"Wacc_g{g}_{it}_{i}")
                        nc.vector.tensor_tensor(W1[:], W_cur[i][:], Wv[:],
                                                ALU.add)
                        W_cur[i] = W1

                c_next = sm_pool.tile([P, GB * LT, C], bf16, tag="cij",
                                      name=f"cij_g{g}_{it + 1}")
                cexp = sm_pool.tile([P, GB * LT, C], f32, tag="cexp",
                                    bufs=1, name=f"cexp_g{g}_{it + 1}")
                csum = sm_pool.tile([P, GB * LT], f32, tag="csum",
                                    name=f"csum_g{g}_{it + 1}")
                crec = sm_pool.tile([P, GB * LT], f32, tag="crec",
                                    name=f"crec_g{g}_{it + 1}")

                for i in range(GB):
                    # deltaT[c, l] = sum_t W_t.T @ u_hatT_t
                    ps_dt = ps_d_pool.tile([C, 512], f32, tag="dd",
                                           name=f"psd_g{g}_{it}_{i}")
                    ps_d = ps_dt[:]
                    for t in range(MT):
                        nc.tensor.matmul(
                            ps_d,
                            W_cur[i][:, t, :],
                            UT[i, t][:],
                            start=(t == 0),
                            stop=(t == MT - 1),
                        )
                    ds = sm_pool.tile([C, 512], f32, tag="ds", bufs=1,
                                      name=f"ds_g{g}_{it}_{i}")
                    nc.vector.tensor_copy(ds[:], ps_d)
                    ps_t = ps_t_pool.tile(
                        [P, LT, C], f32, tag="dt",
                        name=f"pst_g{g}_{it}_{i}")
                    for lt in range(LT):
                        nc.tensor.matmul(
                            ps_t[:, lt, :],
                            ds[:, lt * P:(lt + 1) * P],
                            identf[:],
                            is_transpose=True,
                            start=(lt == 0),
                            stop=(lt == LT - 1),
                        )
                    # per-batch softmax for the next iteration; ps_t IS b_ij
                    nc.scalar.activation(
                        cexp[:, i * LT:(i + 1) * LT, :],
                        ps_t[:], AF.Exp,
                    )
                    nc.vector.tensor_reduce(
                        csum[:, i * LT:(i + 1) * LT],
                        cexp[:, i * LT:(i + 1) * LT, :],
                        AX.X, ALU.add)
                    nc.vector.reciprocal(
                        crec[:, i * LT:(i + 1) * LT],
                        csum[:, i * LT:(i + 1) * LT])
                    nc.vector.tensor_tensor(
                        c_next[:, i * LT:(i + 1) * LT, :],
                        cexp[:, i * LT:(i + 1) * LT, :],
                        crec[:, i * LT:(i + 1) * LT].unsqueeze(2)
                            .broadcast_to((P, LT, C)),
                        ALU.mult,
                    )
                c_cur[0] = c_next

            # last iteration: no b_ij update
            v_all = s_pass_and_squash(ITERS - 1, c_cur_get)
            nc.scalar.dma_start(out_d[g * SP:(g + 1) * SP, :], v_all[:])

    nc.compile()
    return nc


_NC_CACHE = None


def _get_nc():
    global _NC_CACHE
    if _NC_CACHE is None:
        _NC_CACHE = build_kernel()
    return _NC_CACHE


def _make_consts():
    ident = np.eye(P, dtype=_BF16)
    identf = np.eye(C, dtype=np.float32)
    identf128 = np.eye(P, dtype=np.float32)
    cc = np.arange(C)
    tt = np.arange(MT)
    m0u_row = (cc[None, :] == 2 * tt[:, None]).astype(np.float32).reshape(1, MT * C)
    m0l_row = (cc[None, :] == 2 * tt[:, None] + 1).astype(np.float32).reshape(1, MT * C)
    m0 = np.concatenate([
        np.broadcast_to(m0u_row, (O, MT * C)),
        np.broadcast_to(m0l_row, (O, MT * C)),
    ], axis=0).astype(_BF16)
    maskx = (cc[None, :] == (np.arange(P) % C)[:, None]).astype(np.float32)
    return ident, identf, identf128, m0, maskx


def kernel(inputs, fc_w, fc_b, _trace=False):
    from concourse.bass_utils import run_bass_kernel_spmd

    if _trace:
        _install_ntff_shim()

    nc = _get_nc()

    ident, identf, identf128, m0, maskx = _make_consts()
    w_bf = np.asarray(fc_w, dtype=np.float32).astype(_BF16)
    bias_f = np.asarray(fc_b, dtype=np.float32)
    bias_t = np.ascontiguousarray(bias_f.reshape(MT, P).T)
    bias16 = np.broadcast_to(16.0 * bias_f, (BPC, CO)).astype(_BF16)
    x_f = np.asarray(inputs, dtype=np.float32)
    xt_all = np.ascontiguousarray(x_f.transpose(0, 2, 1)).astype(_BF16)
    xsum_all = x_f.sum(axis=1)  # (B, D)

    in_maps = []
    for core in range(NCORES):
        xsumt = np.ascontiguousarray(
            xsum_all[core * BPC:(core + 1) * BPC].T).astype(_BF16)
        in_maps.append({
            "xt": xt_all[core * BPC:(core + 1) * BPC],
            "w": w_bf,
            "bias_t": bias_t,
            "xsumt": xsumt,
            "bias16": bias16,
            "ident": ident,
            "identf": identf,
            "identf128": identf128,
            "m0": m0,
            "mask_x": maskx,
        })

    res = run_bass_kernel_spmd(
        nc, in_maps, core_ids=list(range(NCORES)), trace=_trace,
    )
    out = np.concatenate(
        [res.results[core]["v"].reshape(BPC, C, O) for core in range(NCORES)],
        axis=0,
    )
    if _trace:
        kernel.last_exec_time_ns = res.exec_time_ns
        kernel.last_results = res
    return out


# revision 23
# speedup vs baseline: 1.5962x; 1.5962x over previous
"""Trainium2 Bass kernel for nn_CapsuleLayer (B=64, L=512, D=1024, C=32, O=64).

Strategy: data-parallel over batch across 8 NeuronCores (8 batch elements per
core), processed in 4 pipelined groups of 2 so that projection of group k+1
overlaps routing of group k on the PE. Per core, per batch element:
  u_hatT[co, l] = fc_w.T @ xT (+ fc_b)      PE, fc_w stationary
  u_hat[l, co]  = transpose(u_hatT)          PE transpose
  routing, all on-chip:
    iter 0 is algebraic: c_ij uniform => s0 = (sum_l x) @ W / 32 + 16*bias,
      computed once for all 8 batch elements with a tiny GEMV at kernel start
    s_j  = diag-extract(c_ij.T @ u_hat)      PE cross-matmul, batches packed
                                             into PSUM partitions, bank-major
                                             with a 2-bank rotation
    v_j  = squash(s_j)                       ACT/DVE
    b_ij = u_hat . (v_0+..+v_t)              PE: 16 accumulating matmuls with
                                             block-diag masked weights built
                                             from the accumulated v (W-acc)

Everything except batch sharding + input transpose/cast + x column-sums
happens on device.
"""

import contextlib
import ctypes
import sys
import types

import numpy as np
import ml_dtypes

B, L, D = 64, 512, 1024
C, O = 32, 64
CO = C * O                  # 2048
ITERS = 3
NCORES = 8
BPC = B // NCORES           # 8 batch elements per core
GB = 2                      # batch elements per routing group
NGRP = BPC // GB            # 4
P = 128
KD = D // P                 # 8 contraction chunks
MT = CO // P                # 16 m-tiles of u_hatT (= capsule pairs)
LT = L // P                 # 4 l-chunks
NBANK = CO // 512           # 4 cross-matmul column banks
SP = GB * C                 # s-cross partitions (64)

_BF16 = ml_dtypes.bfloat16

# ---------------------------------------------------------------------------
# NTFF profiling shim (used when tracing is requested by the test harness)
# ---------------------------------------------------------------------------


def _install_ntff_shim():
    if "antenv.axon_hooks" in sys.modules:
        return
    so_path = "/opt/axon/libaxon_pjrt.so"
    hook = None
    try:
        lib = ctypes.CDLL(so_path)
        if hasattr(lib, "axon_start_nrt_profile"):
            lib.axon_start_nrt_profile.argtypes = [
                ctypes.POINTER(ctypes.c_int64),
                ctypes.c_size_t,
            ]
            lib.axon_start_nrt_profile.restype = ctypes.c_int64
            lib.axon_stop_nrt_profile.argtypes = [ctypes.c_char_p]
            lib.axon_stop_nrt_profile.restype = ctypes.c_int64

            @contextlib.contextmanager
            def hook(output_dir, device_ids):
                import jax

                jax.devices()
                if device_ids:
                    ids = (ctypes.c_int64 * len(device_ids))(*device_ids)
                    rc = lib.axon_start_nrt_profile(ids, len(device_ids))
                else:
                    rc = lib.axon_start_nrt_profile(None, 0)
                if rc != 0:
                    raise RuntimeError(f"axon_start_nrt_profile rc={rc}")
                try:
                    yield
                finally:
                    n = lib.axon_stop_nrt_profile(str(output_dir).encode())
                    if n < 0:
                        raise RuntimeError(f"axon_stop_nrt_profile rc={n}")
    except OSError:
        pass
    mod = types.ModuleType("antenv.axon_hooks")
    mod.get_axon_ntff_profile_hook = lambda: hook
    mod.set_axon_ntff_profile_hook = lambda h: None
    sys.modules["antenv.axon_hooks"] = mod

    import concourse.bass_utils as bu

    bu.upload_artifacts = lambda tmpdir: tmpdir


# ---------------------------------------------------------------------------
# Kernel builder
# ---------------------------------------------------------------------------


def build_kernel():
    import concourse.bacc as bacc
    import concourse.tile as tile
    import concourse.mybir as mybir

    f32 = mybir.dt.float32
    bf16 = mybir.dt.bfloat16
    AF = mybir.ActivationFunctionType
    ALU = mybir.AluOpType
    AX = mybir.AxisListType

    nc = bacc.Bacc("TRN2", target_bir_lowering=False, debug=False)

    xt_d = nc.dram_tensor("xt", [BPC, D, L], bf16, kind="ExternalInput")
    w_d = nc.dram_tensor("w", [D, CO], bf16, kind="ExternalInput")
    bias_d = nc.dram_tensor("bias_t", [P, MT], f32, kind="ExternalInput")
    xsumt_d = nc.dram_tensor("xsumt", [D, BPC], bf16, kind="ExternalInput")
    bias16_d = nc.dram_tensor("bias16", [BPC, CO], bf16, kind="ExternalInput")
    ident_d = nc.dram_tensor("ident", [P, P], bf16, kind="ExternalInput")
    identf_d = nc.dram_tensor("identf", [C, C], f32, kind="ExternalInput")
    identf128_d = nc.dram_tensor("identf128", [P, P], f32, kind="ExternalInput")
    m0_d = nc.dram_tensor("m0", [P, MT * C], bf16, kind="ExternalInput")
    maskx_d = nc.dram_tensor("mask_x", [P, C], f32, kind="ExternalInput")
    out_d = nc.dram_tensor("v", [BPC * C, O], f32, kind="ExternalOutput")

    with tile.TileContext(nc) as tc, contextlib.ExitStack() as glb:
        const_pool = glb.enter_context(tc.tile_pool(name="consts", bufs=1))
        w_pool = glb.enter_context(tc.tile_pool(name="w", bufs=KD))
        gscr = glb.enter_context(tc.tile_pool(name="gscr", bufs=1))
        xt_pool = glb.enter_context(tc.tile_pool(name="xt", bufs=2 * KD))
        ut_pool = glb.enter_context(tc.tile_pool(name="ut", bufs=2 * GB * MT))
        u_pool = glb.enter_context(tc.tile_pool(name="u", bufs=2 * GB * LT - 1))
        sm_pool = glb.enter_context(tc.tile_pool(name="sm", bufs=2))
        pp_mm = glb.enter_context(tc.tile_pool(name="ppmm", bufs=3, space="PSUM"))
        pp_tr = glb.enter_context(tc.tile_pool(name="pptr", bufs=1, space="PSUM"))
        ps_s_pool = glb.enter_context(tc.tile_pool(name="pss", bufs=2, space="PSUM"))
        ps_d_pool = glb.enter_context(tc.tile_pool(name="psd", bufs=1, space="PSUM"))
        ps_t_pool = glb.enter_context(tc.tile_pool(name="pst", bufs=1, space="PSUM"))

        # --- constants; g0 xt DMAs issued early so the PE can start ---
        w_sb = []
        for k in range(KD):
            wt = w_pool.tile([P, CO], bf16, tag="w", name=f"w{k}")
            nc.sync.dma_start(wt[:], w_d[k * P:(k + 1) * P, :])
            w_sb.append(wt)
        xt_pre = {}
        for i in range(GB):
            for k in range(KD):
                t = xt_pool.tile([P, L], bf16, tag="xt", name=f"xt_g0_{i}_{k}")
                nc.sync.dma_start(t[:], xt_d[i, k * P:(k + 1) * P, :])
                xt_pre[i, k] = t
        ident = const_pool.tile([P, P], bf16, name="ident")
        nc.sync.dma_start(ident[:], ident_d[:])
        bias_sb = const_pool.tile([P, MT], f32, name="bias_sb")
        nc.sync.dma_start(bias_sb[:], bias_d[:])
        identf = const_pool.tile([C, C], f32, name="identf")
        nc.scalar.dma_start(identf[:], identf_d[:])
        identf128 = const_pool.tile([P, P], f32, name="identf128")
        nc.scalar.dma_start(identf128[:], identf128_d[:])
        m0 = const_pool.tile([P, MT * C], bf16, name="m0")
        nc.scalar.dma_start(m0[:], m0_d[:])
        maskx = const_pool.tile([P, C], f32, name="maskx")
        nc.scalar.dma_start(maskx[:], maskx_d[:])
        eps_sb = const_pool.tile([P, 1], f32, name="eps_sb")
        nc.vector.memset(eps_sb[:], 1e-8)
        # ttv[(j,o), m, b]: partition j*64+o holds v0[b, c=2m+j, o]
        ttv = const_pool.tile([P, MT, BPC], bf16, name="ttv")

        # --- iter-0 GEMV: s0 = xsum @ W / 32 + 16*bias for all 8 elems.
        # Issued into the PE queue between the two g0 projections; the tiny
        # squash chain runs on DVE/ACT under the second projection.
        xs_sb = gscr.tile([P, KD, BPC], bf16, name="xs_sb")
        nc.sync.dma_start(
            xs_sb[:], xsumt_d[:].rearrange("(k p) b -> p k b", p=P))
        bias16_sb = gscr.tile([BPC, CO], bf16, tag="g8a", name="bias16")
        nc.sync.dma_start(bias16_sb[:], bias16_d[:])
        v0_sb = gscr.tile([BPC, CO], bf16, name="v0_sb")

        def gemv_mm():
            s0_sb = gscr.tile([BPC, CO], bf16, name="s0_sb")
            for n in range(NBANK):
                ps0 = pp_mm.tile([BPC, 512], f32, tag="mm", name=f"ps0_{n}")
                for k in range(KD):
                    nc.tensor.matmul(
                        ps0[:], xs_sb[:, k, :],
                        w_sb[k][:, n * 512:(n + 1) * 512],
                        start=(k == 0), stop=(k == KD - 1),
                    )
                nc.vector.scalar_tensor_tensor(
                    s0_sb[:, n * 512:(n + 1) * 512], ps0[:], 1.0 / 32.0,
                    bias16_sb[:, n * 512:(n + 1) * 512], ALU.mult, ALU.add,
                )
            # squash s0 -> v0  (scale varies along free dim (c))
            tmp0 = gscr.tile([BPC, CO], bf16, tag="g8a", name="tmp0")
            nc.vector.tensor_tensor(tmp0[:], s0_sb[:], s0_sb[:], ALU.mult)
            sq0 = gscr.tile([BPC, C], f32, name="sq0")
            nc.vector.tensor_reduce(
                sq0[:], tmp0[:].rearrange("b (c o) -> b c o", c=C),
                AX.X, ALU.add)
            r10 = gscr.tile([BPC, C], f32, name="r10")
            nc.scalar.activation(r10[:], sq0[:], AF.Sqrt, bias=eps_sb[:BPC])
            r20 = gscr.tile([BPC, C], f32, name="r20")
            nc.vector.scalar_tensor_tensor(
                r20[:], sq0[:], 1.0, r10[:], ALU.add, ALU.mult)
            rr0 = gscr.tile([BPC, C], f32, name="rr0")
            nc.vector.reciprocal(rr0[:], r20[:])
            sc0 = gscr.tile([BPC, C], f32, name="sc0")
            nc.vector.tensor_tensor(sc0[:], sq0[:], rr0[:], ALU.mult)
            nc.vector.tensor_tensor(
                v0_sb[:].rearrange("b (c o) -> b c o", c=C),
                s0_sb[:].rearrange("b (c o) -> b c o", c=C),
                sc0[:].unsqueeze(2).broadcast_to((BPC, C, O)),
                ALU.mult,
            )

        def gemv_tt():
            tt_ps = ps_t_pool.tile([P, MT * BPC], bf16, tag="dt",
                                   name="tt_ps")
            for m in range(MT):
                nc.tensor.matmul(
                    tt_ps[:, m * BPC:(m + 1) * BPC],
                    v0_sb[:, m * P:(m + 1) * P],
                    ident[:BPC, :BPC],
                    is_transpose=True,
                    start=(m == 0), stop=(m == MT - 1),
                )
            nc.vector.tensor_copy(
                ttv[:], tt_ps[:].rearrange("p (m b) -> p m b", m=MT))

        # ------------------------------------------------------------------
        # Software-pipelined schedule: the PE queue interleaves projection
        # m-tiles of group g with the routing stages of group g-1 so that
        # routing's serial DVE/ACT chains never leave the PE idle.
        # ------------------------------------------------------------------

        def make_u(g):
            U = {}
            for i in range(GB):
                for lt in range(LT):
                    U[i, lt] = u_pool.tile([P, CO], bf16, tag="u",
                                           name=f"u_g{g}_{i}_{lt}")
            return U, {}

        def proj_steps(g, U, UT, xt_sb):
            def dma_step(i, b):
                def f():
                    for k in range(KD):
                        t = xt_pool.tile([P, L], bf16, tag="xt",
                                         name=f"xt_g{g}_{i}_{k}")
                        nc.sync.dma_start(t[:], xt_d[b, k * P:(k + 1) * P, :])
                        xt_sb[i, k] = t
                return f

            pcell = [None]

            def m_step(i, m):
                def f():
                    ps = pp_mm.tile([P, 512], f32, tag="mm",
                                    name=f"ps_g{g}_{m}_{i}")
                    for k in range(KD):
                        nc.tensor.matmul(
                            ps[:],
                            w_sb[k][:, m * P:(m + 1) * P],
                            xt_sb[i, k][:],
                            start=(k == 0),
                            stop=(k == KD - 1),
                        )
                    ut = ut_pool.tile([P, L], bf16, tag="ut",
                                      name=f"ut_g{g}_{i}_{m}")
                    nc.scalar.activation(
                        ut[:], ps[:], AF.Identity,
                        bias=bias_sb[:, m:m + 1],
                    )
                    UT[i, m] = ut
                    mh = m % 2
                    if mh == 0:
                        pcell[0] = pp_tr.tile([P, LT, 2, P], bf16, tag="tr",
                                              name=f"ptr_g{g}_{m}_{i}")
                    ptr = pcell[0]
                    for lt in range(LT):
                        nc.tensor.matmul(
                            ptr[:, lt, mh, :],
                            ut[:, lt * P:(lt + 1) * P],
                            ident[:],
                            is_transpose=True,
                            start=(mh == 0 and lt == 0),
                            stop=(mh == 1 and lt == LT - 1),
                        )
                    if mh == 1:
                        for lt in range(LT):
                            nc.vector.tensor_copy(
                                U[i, lt][:, (m - 1) * P:(m + 1) * P],
                                ptr[:, lt, :, :],
                            )
                return f

            def pair_step(i, m):
                a, b2 = m_step(i, m), m_step(i, m + 1)

                def f():
                    a()
                    b2()
                return f

            dmas, ms = [], []
            for i in range(GB):
                b = g * GB + i
                if g == 0:
                    for k in range(KD):
                        xt_sb[i, k] = xt_pre[i, k]
                else:
                    dmas.append(dma_step(i, b))
                # issue m-tiles in pairs so the shared transpose-PSUM
                # accumulation group is never split by interleaved stages
                for m in range(0, MT, 2):
                    ms.append(pair_step(i, m))
            return dmas, ms

        def routing_stages(g, U, UT):
            bs = [g * GB + i for i in range(GB)]
            st = {}
            W_cur = {}
            c_cur = {}

            def c_get(i, lt):
                return c_cur["t"][:, i * LT + lt, :]

            def w_from_ttv():
                for i in range(GB):
                    W0 = sm_pool.tile([P, MT, C], bf16, tag=f"W{i}",
                                      name=f"W_g{g}_0_{i}")
                    nc.vector.tensor_tensor(
                        W0[:O],
                        ttv[:O, :, bs[i]].unsqueeze(2)
                            .broadcast_to((O, MT, C)),
                        m0[:O].rearrange("p (t c) -> p t c", t=MT),
                        ALU.mult,
                    )
                    nc.vector.tensor_tensor(
                        W0[O:],
                        ttv[O:, :, bs[i]].unsqueeze(2)
                            .broadcast_to((O, MT, C)),
                        m0[O:].rearrange("p (t c) -> p t c", t=MT),
                        ALU.mult,
                    )
                    W_cur[i] = W0

            def spass_bank(it, n):
                def f():
                    if n == 0:
                        st["sps", it] = []
                        st["s01", it] = sm_pool.tile([SP, O], f32, tag="s01",
                                                     name=f"s01_g{g}_{it}")
                        st["s23", it] = sm_pool.tile([SP, O], f32, tag="s23",
                                                     name=f"s23_g{g}_{it}")
                    sps = st["sps", it]
                    ps_s = ps_s_pool.tile([SP, 512], f32, tag="ss",
                                          name=f"pss_g{g}_{it}_{n}")
                    for lt in range(LT):
                        for i in range(GB):
                            nc.tensor.matmul(
                                ps_s[i * C:(i + 1) * C, :],
                                c_get(i, lt),
                                U[i, lt][:, n * 512:(n + 1) * 512],
                                start=(lt == 0),
                                stop=(lt == LT - 1),
                                tile_position=(0, i * C),
                                skip_group_check=(i > 0),
                            )
                    tmpb = sm_pool.tile([SP, 512], f32, tag="tmpb",
                                        name=f"tmpb_g{g}_{it}_{n}")
                    nc.vector.tensor_tensor(
                        tmpb[:].rearrange("p (c o) -> p c o", c=8),
                        ps_s[:].rearrange("p (c o) -> p c o", c=8),
                        maskx[:SP, n * 8:(n + 1) * 8].unsqueeze(2)
                            .broadcast_to((SP, 8, O)),
                        ALU.mult,
                    )
                    sp = sm_pool.tile([SP, O], f32, tag=f"spart{n}", bufs=1,
                                      name=f"sp_g{g}_{it}_{n}")
                    nc.vector.tensor_reduce(
                        sp[:],
                        tmpb[:].rearrange("p (c o) -> p o c", c=8),
                        AX.X, ALU.add,
                    )
                    sps.append(sp)
                    if n == 1:
                        nc.vector.tensor_tensor(
                            st["s01", it][:], sps[0][:], sps[1][:], ALU.add)
                    elif n == 3:
                        nc.vector.tensor_tensor(
                            st["s23", it][:], sps[2][:], sps[3][:], ALU.add)
                return f

            def squash(it, last):
                def f():
                    s_all = sm_pool.tile([SP, O], f32, tag="sall",
                                         name=f"sall_g{g}_{it}")
                    nc.vector.tensor_tensor(s_all[:], st["s01", it][:],
                                            st["s23", it][:], ALU.add)
                    # squash: v = s * sq/(1+sq)/sqrt(sq+1e-8), sq = sum s^2
                    ssq = sm_pool.tile([SP, O], f32, tag="ssq", bufs=1,
                                       name=f"ssq_g{g}_{it}")
                    sq = sm_pool.tile([SP, 1], f32, tag="sq",
                                      name=f"sq_g{g}_{it}")
                    nc.scalar.activation(ssq[:], s_all[:], AF.Square,
                                         accum_out=sq[:])
                    r1 = sm_pool.tile([SP, 1], f32, tag="r1",
                                      name=f"r1_g{g}_{it}")
                    nc.scalar.activation(r1[:], sq[:], AF.Sqrt,
                                         bias=eps_sb[:SP])
                    r2 = sm_pool.tile([SP, 1], f32, tag="r2",
                                      name=f"r2_g{g}_{it}")
                    nc.vector.scalar_tensor_tensor(
                        r2[:], sq[:], 1.0, r1[:], ALU.add, ALU.mult,
                    )
                    rr = sm_pool.tile([SP, 1], f32, tag="rr",
                                      name=f"rr_g{g}_{it}")
                    nc.vector.reciprocal(rr[:], r2[:])
                    v_all = sm_pool.tile([SP, O], f32, tag="vall",
                                         name=f"vall_g{g}_{it}")
                    nc.vector.tensor_scalar(
                        v_all[:], s_all[:], sq[:], rr[:], ALU.mult, ALU.mult,
                    )
                    st["v"] = v_all
                    if last:
                        nc.scalar.dma_start(out_d[g * SP:(g + 1) * SP, :],
                                            v_all[:])
                return f

            def psv_wbuild(it):
                def f():
                    ps_v_t = ps_t_pool.tile([O, SP], f32, tag="dt",
                                            name=f"psv_g{g}_{it}")
                    nc.tensor.transpose(ps_v_t[:], st["v"][:],
                                        identf128[:SP, :SP])
                    ps_v = ps_v_t[:]
                    for i in range(GB):
                        Wv = sm_pool.tile([P, MT, C], bf16, tag="Wv",
                                          name=f"Wv_g{g}_{it}_{i}")
                        nc.vector.tensor_tensor(
                            Wv[:O],
                            ps_v[:, i * C:(i + 1) * C:2].unsqueeze(2)
                                .broadcast_to((O, MT, C)),
                            m0[:O].rearrange("p (t c) -> p t c", t=MT),
                            ALU.mult,
                        )
                        nc.vector.tensor_tensor(
                            Wv[O:],
                            ps_v[:, i * C + 1:(i + 1) * C:2].unsqueeze(2)
                                .broadcast_to((O, MT, C)),
                            m0[O:].rearrange("p (t c) -> p t c", t=MT),
                            ALU.mult,
                        )
                        W1 = sm_pool.tile([P, MT, C], bf16, tag=f"W{i}",
                                          name=f"Wacc_g{g}_{it}_{i}")
                        nc.vector.tensor_tensor(W1[:], W_cur[i][:], Wv[:],
                                                ALU.add)
                        W_cur[i] = W1
                return f

            def delta_mm(it, i):
                def f():
                    ps_dt = ps_d_pool.tile([C, 512], f32, tag="dd",
                                           name=f"psd_g{g}_{it}_{i}")
                    st["psd", i] = ps_dt
                    for t in range(MT):
                        nc.tensor.matmul(
                            ps_dt[:],
                            W_cur[i][:, t, :],
                            UT[i, t][:],
                            start=(t == 0),
                            stop=(t == MT - 1),
                        )
                return f

            def delta_fin(it, i):
                def f():
                    if i == 0:
                        st["cnext"] = sm_pool.tile(
                            [P, GB * LT, C], bf16, tag="cij",
                            name=f"cij_g{g}_{it + 1}")
                        st["cexp"] = sm_pool.tile(
                            [P, GB * LT, C], f32, tag="cexp", bufs=1,
                            name=f"cexp_g{g}_{it + 1}")
                        st["csum"] = sm_pool.tile(
                            [P, GB * LT], f32, tag="csum",
                            name=f"csum_g{g}_{it + 1}")
                        st["crec"] = sm_pool.tile(
                            [P, GB * LT], f32, tag="crec",
                            name=f"crec_g{g}_{it + 1}")
                    ds = sm_pool.tile([C, 512], f32, tag="ds", bufs=1,
                                      name=f"ds_g{g}_{it}_{i}")
                    nc.vector.tensor_copy(ds[:], st["psd", i][:])
                    ps_t = ps_t_pool.tile([P, LT, C], f32, tag="dt",
                                          name=f"pst_g{g}_{it}_{i}")
                    for lt in range(LT):
                        nc.tensor.matmul(
                            ps_t[:, lt, :],
                            ds[:, lt * P:(lt + 1) * P],
                            identf[:],
                            is_transpose=True,
                            start=(lt == 0),
                            stop=(lt == LT - 1),
                        )
                    # ps_t IS b_ij for this batch elem; softmax for next iter
                    cexp, csum, crec = st["cexp"], st["csum"], st["crec"]
                    nc.scalar.activation(
                        cexp[:, i * LT:(i + 1) * LT, :], ps_t[:], AF.Exp)
                    nc.vector.tensor_reduce(
                        csum[:, i * LT:(i + 1) * LT],
                        cexp[:, i * LT:(i + 1) * LT, :], AX.X, ALU.add)
                    nc.vector.reciprocal(
                        crec[:, i * LT:(i + 1) * LT],
                        csum[:, i * LT:(i + 1) * LT])
                    nc.vector.tensor_tensor(
                        st["cnext"][:, i * LT:(i + 1) * LT, :],
                        cexp[:, i * LT:(i + 1) * LT, :],
                        crec[:, i * LT:(i + 1) * LT].unsqueeze(2)
                            .broadcast_to((P, LT, C)),
                        ALU.mult,
                    )
                    if i == GB - 1:
                        c_cur["t"] = st["cnext"]
                return f

            stages = [w_from_ttv]
            for i in range(GB):
                stages.append(delta_mm(0, i))
                stages.append(delta_fin(0, i))
            for it in range(1, ITERS):
                for n in range(NBANK):
                    stages.append(spass_bank(it, n))
                stages.append(squash(it, it == ITERS - 1))
                if it < ITERS - 1:
                    stages.append(psv_wbuild(it))
                    for i in range(GB):
                        stages.append(delta_mm(it, i))
                        stages.append(delta_fin(it, i))
            return stages

        INTERLEAVE = False

        def interleave(stages, ms):
            if not INTERLEAVE:
                for s in stages:
                    s()
                for f in ms:
                    f()
                return
            pi = 0
            ns = len(stages)
            for ri, s in enumerate(stages):
                s()
                tgt = min(len(ms), ((ri + 1) * len(ms)) // ns)
                while pi < tgt:
                    ms[pi]()
                    pi += 1
            while pi < len(ms):
                ms[pi]()
                pi += 1

        Us = {}
        Us[0] = make_u(0)
        dmas0, ms0 = proj_steps(0, Us[0][0], Us[0][1], {})
        for f in ms0[:MT]:
            f()
        gemv_mm()
        for f in ms0[MT:]:
            f()
        gemv_tt()
        for g in range(1, NGRP):
            Us[g] = make_u(g)
            dmas, ms = proj_steps(g, Us[g][0], Us[g][1], {})
            for d in dmas:
                d()
            interleave(routing_stages(g - 1, *Us[g - 1]), ms)
            del Us[g - 1]
        for s in routing_stages(NGRP - 1, *Us[NGRP - 1]):
            s()

    nc.compile()
    return nc


_NC_CACHE = None


def _get_nc():
    global _NC_CACHE
    if _NC_CACHE is None:
        _NC_CACHE = build_kernel()
    return _NC_CACHE


def _make_consts():
    ident = np.eye(P, dtype=_BF16)
    identf = np.eye(C, dtype=np.float32)
    identf128 = np.eye(P, dtype=np.float32)
    cc = np.arange(C)
    tt = np.arange(MT)
    m0u_row = (cc[None, :] == 2 * tt[:, None]).astype(np.float32).reshape(1, MT * C)
    m0l_row = (cc[None, :] == 2 * tt[:, None] + 1).astype(np.float32).reshape(1, MT * C)
    m0 = np.concatenate([
        np.broadcast_to(m0u_row, (O, MT * C)),
        np.broadcast_to(m0l_row, (O, MT * C)),
    ], axis=0).astype(_BF16)
    maskx = (cc[None, :] == (np.arange(P) % C)[:, None]).astype(np.float32)
    return ident, identf, identf128, m0, maskx


def kernel(inputs, fc_w, fc_b, _trace=False):
    from concourse.bass_utils import run_bass_kernel_spmd

    if _trace:
        _install_ntff_shim()

    nc = _get_nc()

    ident, identf, identf128, m0, maskx = _make_consts()
    w_bf = np.asarray(fc_w, dtype=np.float32).astype(_BF16)
    bias_f = np.asarray(fc_b, dtype=np.float32)
    bias_t = np.ascontiguousarray(bias_f.reshape(MT, P).T)
    bias16 = np.broadcast_to(16.0 * bias_f, (BPC, CO)).astype(_BF16)
    x_f = np.asarray(inputs, dtype=np.float32)
    xt_all = np.ascontiguousarray(x_f.transpose(0, 2, 1)).astype(_BF16)
    xsum_all = x_f.sum(axis=1)  # (B, D)

    in_maps = []
    for core in range(NCORES):
        xsumt = np.ascontiguousarray(
            xsum_all[core * BPC:(core + 1) * BPC].T).astype(_BF16)
        in_maps.append({
            "xt": xt_all[core * BPC:(core + 1) * BPC],
            "w": w_bf,
            "bias_t": bias_t,
            "xsumt": xsumt,
            "bias16": bias16,
            "ident": ident,
            "identf": identf,
            "identf128": identf128,
            "m0": m0,
            "mask_x": maskx,
        })

    res = run_bass_kernel_spmd(
        nc, in_maps, core_ids=list(range(NCORES)), trace=_trace,
    )
    out = np.concatenate(
        [res.results[core]["v"].reshape(BPC, C, O) for core in range(NCORES)],
        axis=0,
    )
    if _trace:
        kernel.last_exec_time_ns = res.exec_time_ns
        kernel.last_results = res
    return out


# revision 24
# speedup vs baseline: 1.6499x; 1.0336x over previous
"""Trainium2 Bass kernel for nn_CapsuleLayer (B=64, L=512, D=1024, C=32, O=64).

Strategy: data-parallel over batch across 8 NeuronCores (8 batch elements per
core), processed in 4 pipelined groups of 2 so that projection of group k+1
overlaps routing of group k on the PE. Per core, per batch element:
  u_hatT[co, l] = fc_w.T @ xT (+ fc_b)      PE, fc_w stationary
  u_hat[l, co]  = transpose(u_hatT)          PE transpose
  routing, all on-chip:
    iter 0 is algebraic: c_ij uniform => s0 = (sum_l x) @ W / 32 + 16*bias,
      computed once for all 8 batch elements with a tiny GEMV at kernel start
    s_j  = diag-extract(c_ij.T @ u_hat)      PE cross-matmul, batches packed
                                             into PSUM partitions, bank-major
                                             with a 2-bank rotation
    v_j  = squash(s_j)                       ACT/DVE
    b_ij = u_hat . (v_0+..+v_t)              PE: 16 accumulating matmuls with
                                             block-diag masked weights built
                                             from the accumulated v (W-acc)

Everything except batch sharding + input transpose/cast + x column-sums
happens on device.
"""

import contextlib
import ctypes
import sys
import types

import numpy as np
import ml_dtypes

B, L, D = 64, 512, 1024
C, O = 32, 64
CO = C * O                  # 2048
ITERS = 3
NCORES = 8
BPC = B // NCORES           # 8 batch elements per core
GB = 2                      # batch elements per routing group
NGRP = BPC // GB            # 4
P = 128
KD = D // P                 # 8 contraction chunks
MT = CO // P                # 16 m-tiles of u_hatT (= capsule pairs)
LT = L // P                 # 4 l-chunks
NBANK = CO // 512           # 4 cross-matmul column banks
SP = GB * C                 # s-cross partitions (64)

_BF16 = ml_dtypes.bfloat16

# ---------------------------------------------------------------------------
# NTFF profiling shim (used when tracing is requested by the test harness)
# ---------------------------------------------------------------------------


def _install_ntff_shim():
    if "antenv.axon_hooks" in sys.modules:
        return
    so_path = "/opt/axon/libaxon_pjrt.so"
    hook = None
    try:
        lib = ctypes.CDLL(so_path)
        if hasattr(lib, "axon_start_nrt_profile"):
            lib.axon_start_nrt_profile.argtypes = [
                ctypes.POINTER(ctypes.c_int64),
                ctypes.c_size_t,
            ]
            lib.axon_start_nrt_profile.restype = ctypes.c_int64
            lib.axon_stop_nrt_profile.argtypes = [ctypes.c_char_p]
            lib.axon_stop_nrt_profile.restype = ctypes.c_int64

            @contextlib.contextmanager
            def hook(output_dir, device_ids):
                import jax

                jax.devices()
                if device_ids:
                    ids = (ctypes.c_int64 * len(device_ids))(*device_ids)
                    rc = lib.axon_start_nrt_profile(ids, len(device_ids))
                else:
                    rc = lib.axon_start_nrt_profile(None, 0)
                if rc != 0:
                    raise RuntimeError(f"axon_start_nrt_profile rc={rc}")
                try:
                    yield
                finally:
                    n = lib.axon_stop_nrt_profile(str(output_dir).encode())
                    if n < 0:
                        raise RuntimeError(f"axon_stop_nrt_profile rc={n}")
    except OSError:
        pass
    mod = types.ModuleType("antenv.axon_hooks")
    mod.get_axon_ntff_profile_hook = lambda: hook
    mod.set_axon_ntff_profile_hook = lambda h: None
    sys.modules["antenv.axon_hooks"] = mod

    import concourse.bass_utils as bu

    bu.upload_artifacts = lambda tmpdir: tmpdir


# ---------------------------------------------------------------------------
# Kernel builder
# ---------------------------------------------------------------------------


def build_kernel():
    import concourse.bacc as bacc
    import concourse.tile as tile
    import concourse.mybir as mybir

    f32 = mybir.dt.float32
    bf16 = mybir.dt.bfloat16
    AF = mybir.ActivationFunctionType
    ALU = mybir.AluOpType
    AX = mybir.AxisListType

    nc = bacc.Bacc("TRN2", target_bir_lowering=False, debug=False)

    xt_d = nc.dram_tensor("xt", [BPC, D, L], bf16, kind="ExternalInput")
    w_d = nc.dram_tensor("w", [D, CO], bf16, kind="ExternalInput")
    bias_d = nc.dram_tensor("bias_t", [P, MT], f32, kind="ExternalInput")
    xsumt_d = nc.dram_tensor("xsumt", [D, BPC], bf16, kind="ExternalInput")
    bias16_d = nc.dram_tensor("bias16", [BPC, CO], bf16, kind="ExternalInput")
    ident_d = nc.dram_tensor("ident", [P, P], bf16, kind="ExternalInput")
    identf_d = nc.dram_tensor("identf", [C, C], f32, kind="ExternalInput")
    identf128_d = nc.dram_tensor("identf128", [P, P], f32, kind="ExternalInput")
    m0_d = nc.dram_tensor("m0", [P, MT * C], bf16, kind="ExternalInput")
    maskx_d = nc.dram_tensor("mask_x", [P, C], f32, kind="ExternalInput")
    out_d = nc.dram_tensor("v", [BPC * C, O], f32, kind="ExternalOutput")

    with tile.TileContext(nc) as tc, contextlib.ExitStack() as glb:
        const_pool = glb.enter_context(tc.tile_pool(name="consts", bufs=1))
        w_pool = glb.enter_context(tc.tile_pool(name="w", bufs=KD))
        gscr = glb.enter_context(tc.tile_pool(name="gscr", bufs=1))
        xt_pool = glb.enter_context(tc.tile_pool(name="xt", bufs=2 * KD))
        ut_pool = glb.enter_context(tc.tile_pool(name="ut", bufs=2 * GB * MT))
        u_pool = glb.enter_context(tc.tile_pool(name="u", bufs=2 * GB * LT - 1))
        sm_pool = glb.enter_context(tc.tile_pool(name="sm", bufs=2))
        pp_mm = glb.enter_context(tc.tile_pool(name="ppmm", bufs=3, space="PSUM"))
        pp_tr = glb.enter_context(tc.tile_pool(name="pptr", bufs=1, space="PSUM"))
        ps_s_pool = glb.enter_context(tc.tile_pool(name="pss", bufs=2, space="PSUM"))
        ps_d_pool = glb.enter_context(tc.tile_pool(name="psd", bufs=1, space="PSUM"))
        ps_t_pool = glb.enter_context(tc.tile_pool(name="pst", bufs=1, space="PSUM"))

        # --- constants; g0 xt DMAs issued early so the PE can start ---
        w_sb = []
        for k in range(KD):
            wt = w_pool.tile([P, CO], bf16, tag="w", name=f"w{k}")
            nc.sync.dma_start(wt[:], w_d[k * P:(k + 1) * P, :])
            w_sb.append(wt)
        xt_pre = {}
        for i in range(GB):
            for k in range(KD):
                t = xt_pool.tile([P, L], bf16, tag="xt", name=f"xt_g0_{i}_{k}")
                nc.sync.dma_start(t[:], xt_d[i, k * P:(k + 1) * P, :])
                xt_pre[i, k] = t
        ident = const_pool.tile([P, P], bf16, name="ident")
        nc.sync.dma_start(ident[:], ident_d[:])
        bias_sb = const_pool.tile([P, MT], f32, name="bias_sb")
        nc.sync.dma_start(bias_sb[:], bias_d[:])
        identf = const_pool.tile([C, C], f32, name="identf")
        nc.scalar.dma_start(identf[:], identf_d[:])
        identf128 = const_pool.tile([P, P], f32, name="identf128")
        nc.scalar.dma_start(identf128[:], identf128_d[:])
        m0 = const_pool.tile([P, MT * C], bf16, name="m0")
        nc.scalar.dma_start(m0[:], m0_d[:])
        maskx = const_pool.tile([P, C], f32, name="maskx")
        nc.scalar.dma_start(maskx[:], maskx_d[:])
        eps_sb = const_pool.tile([P, 1], f32, name="eps_sb")
        nc.vector.memset(eps_sb[:], 1e-8)
        # ttv[(j,o), m, b]: partition j*64+o holds v0[b, c=2m+j, o]
        ttv = const_pool.tile([P, MT, BPC], bf16, name="ttv")

        # --- iter-0 GEMV: s0 = xsum @ W / 32 + 16*bias for all 8 elems.
        # Issued into the PE queue between the two g0 projections; the tiny
        # squash chain runs on DVE/ACT under the second projection.
        xs_sb = gscr.tile([P, KD, BPC], bf16, name="xs_sb")
        nc.sync.dma_start(
            xs_sb[:], xsumt_d[:].rearrange("(k p) b -> p k b", p=P))
        bias16_sb = gscr.tile([BPC, CO], bf16, tag="g8a", name="bias16")
        nc.sync.dma_start(bias16_sb[:], bias16_d[:])
        v0_sb = gscr.tile([BPC, CO], bf16, name="v0_sb")

        def gemv_mm():
            s0_sb = gscr.tile([BPC, CO], bf16, name="s0_sb")
            for n in range(NBANK):
                ps0 = pp_mm.tile([BPC, 512], f32, tag="mm", name=f"ps0_{n}")
                for k in range(KD):
                    nc.tensor.matmul(
                        ps0[:], xs_sb[:, k, :],
                        w_sb[k][:, n * 512:(n + 1) * 512],
                        start=(k == 0), stop=(k == KD - 1),
                    )
                nc.vector.scalar_tensor_tensor(
                    s0_sb[:, n * 512:(n + 1) * 512], ps0[:], 1.0 / 32.0,
                    bias16_sb[:, n * 512:(n + 1) * 512], ALU.mult, ALU.add,
                )
            # squash s0 -> v0  (scale varies along free dim (c))
            tmp0 = gscr.tile([BPC, CO], bf16, tag="g8a", name="tmp0")
            nc.vector.tensor_tensor(tmp0[:], s0_sb[:], s0_sb[:], ALU.mult)
            sq0 = gscr.tile([BPC, C], f32, name="sq0")
            nc.vector.tensor_reduce(
                sq0[:], tmp0[:].rearrange("b (c o) -> b c o", c=C),
                AX.X, ALU.add)
            r10 = gscr.tile([BPC, C], f32, name="r10")
            nc.scalar.activation(r10[:], sq0[:], AF.Sqrt, bias=eps_sb[:BPC])
            r20 = gscr.tile([BPC, C], f32, name="r20")
            nc.vector.scalar_tensor_tensor(
                r20[:], sq0[:], 1.0, r10[:], ALU.add, ALU.mult)
            rr0 = gscr.tile([BPC, C], f32, name="rr0")
            nc.vector.reciprocal(rr0[:], r20[:])
            sc0 = gscr.tile([BPC, C], f32, name="sc0")
            nc.vector.tensor_tensor(sc0[:], sq0[:], rr0[:], ALU.mult)
            nc.vector.tensor_tensor(
                v0_sb[:].rearrange("b (c o) -> b c o", c=C),
                s0_sb[:].rearrange("b (c o) -> b c o", c=C),
                sc0[:].unsqueeze(2).broadcast_to((BPC, C, O)),
                ALU.mult,
            )

        def gemv_tt():
            tt_ps = ps_t_pool.tile([P, MT * BPC], bf16, tag="dt",
                                   name="tt_ps")
            for m in range(MT):
                nc.tensor.matmul(
                    tt_ps[:, m * BPC:(m + 1) * BPC],
                    v0_sb[:, m * P:(m + 1) * P],
                    ident[:BPC, :BPC],
                    is_transpose=True,
                    start=(m == 0), stop=(m == MT - 1),
                )
            nc.vector.tensor_copy(
                ttv[:], tt_ps[:].rearrange("p (m b) -> p m b", m=MT))

        # ------------------------------------------------------------------
        # Software-pipelined schedule: the PE queue interleaves projection
        # m-tiles of group g with the routing stages of group g-1 so that
        # routing's serial DVE/ACT chains never leave the PE idle.
        # ------------------------------------------------------------------

        def make_u(g):
            U = {}
            for i in range(GB):
                for lt in range(LT):
                    U[i, lt] = u_pool.tile([P, CO], bf16, tag="u",
                                           name=f"u_g{g}_{i}_{lt}")
            return U, {}

        def proj_steps(g, U, UT, xt_sb):
            def dma_step(i, b):
                def f():
                    for k in range(KD):
                        t = xt_pool.tile([P, L], bf16, tag="xt",
                                         name=f"xt_g{g}_{i}_{k}")
                        nc.sync.dma_start(t[:], xt_d[b, k * P:(k + 1) * P, :])
                        xt_sb[i, k] = t
                return f

            pcell = [None]

            def m_step(i, m):
                def f():
                    ps = pp_mm.tile([P, 512], f32, tag="mm",
                                    name=f"ps_g{g}_{m}_{i}")
                    for k in range(KD):
                        nc.tensor.matmul(
                            ps[:],
                            w_sb[k][:, m * P:(m + 1) * P],
                            xt_sb[i, k][:],
                            start=(k == 0),
                            stop=(k == KD - 1),
                        )
                    ut = ut_pool.tile([P, L], bf16, tag="ut",
                                      name=f"ut_g{g}_{i}_{m}")
                    nc.scalar.activation(
                        ut[:], ps[:], AF.Identity,
                        bias=bias_sb[:, m:m + 1],
                    )
                    UT[i, m] = ut
                    mh = m % 2
                    if mh == 0:
                        pcell[0] = pp_tr.tile([P, LT, 2, P], bf16, tag="tr",
                                              name=f"ptr_g{g}_{m}_{i}")
                    ptr = pcell[0]
                    for lt in range(LT):
                        nc.tensor.matmul(
                            ptr[:, lt, mh, :],
                            ut[:, lt * P:(lt + 1) * P],
                            ident[:],
                            is_transpose=True,
                            start=(mh == 0 and lt == 0),
                            stop=(mh == 1 and lt == LT - 1),
                        )
                    if mh == 1:
                        for lt in range(LT):
                            nc.vector.tensor_copy(
                                U[i, lt][:, (m - 1) * P:(m + 1) * P],
                                ptr[:, lt, :, :],
                            )
                return f

            def pair_step(i, m):
                a, b2 = m_step(i, m), m_step(i, m + 1)

                def f():
                    a()
                    b2()
                return f

            dmas, ms = [], []
            for i in range(GB):
                b = g * GB + i
                if g == 0:
                    for k in range(KD):
                        xt_sb[i, k] = xt_pre[i, k]
                else:
                    dmas.append(dma_step(i, b))
                # issue m-tiles in pairs so the shared transpose-PSUM
                # accumulation group is never split by interleaved stages
                for m in range(0, MT, 2):
                    ms.append(pair_step(i, m))
            return dmas, ms

        def routing_stages(g, U, UT):
            bs = [g * GB + i for i in range(GB)]
            st = {}
            W_cur = {}
            c_cur = {}

            def c_get(i, lt):
                return c_cur["t"][:, i * LT + lt, :]

            def w_from_ttv():
                for i in range(GB):
                    W0 = sm_pool.tile([P, MT, C], bf16, tag=f"W{i}",
                                      name=f"W_g{g}_0_{i}")
                    nc.vector.tensor_tensor(
                        W0[:O],
                        ttv[:O, :, bs[i]].unsqueeze(2)
                            .broadcast_to((O, MT, C)),
                        m0[:O].rearrange("p (t c) -> p t c", t=MT),
                        ALU.mult,
                    )
                    nc.vector.tensor_tensor(
                        W0[O:],
                        ttv[O:, :, bs[i]].unsqueeze(2)
                            .broadcast_to((O, MT, C)),
                        m0[O:].rearrange("p (t c) -> p t c", t=MT),
                        ALU.mult,
                    )
                    W_cur[i] = W0

            def spass_bank(it, n):
                def f():
                    if n == 0:
                        st["sps", it] = []
                        st["s01", it] = sm_pool.tile([SP, O], f32, tag="s01",
                                                     name=f"s01_g{g}_{it}")
                        st["s23", it] = sm_pool.tile([SP, O], f32, tag="s23",
                                                     name=f"s23_g{g}_{it}")
                    sps = st["sps", it]
                    ps_s = ps_s_pool.tile([SP, 512], f32, tag="ss",
                                          name=f"pss_g{g}_{it}_{n}")
                    for lt in range(LT):
                        for i in range(GB):
                            nc.tensor.matmul(
                                ps_s[i * C:(i + 1) * C, :],
                                c_get(i, lt),
                                U[i, lt][:, n * 512:(n + 1) * 512],
                                start=(lt == 0),
                                stop=(lt == LT - 1),
                                tile_position=(0, i * C),
                                skip_group_check=(i > 0),
                            )
                    tmpb = sm_pool.tile([SP, 512], f32, tag="tmpb",
                                        name=f"tmpb_g{g}_{it}_{n}")
                    nc.vector.tensor_tensor(
                        tmpb[:].rearrange("p (c o) -> p c o", c=8),
                        ps_s[:].rearrange("p (c o) -> p c o", c=8),
                        maskx[:SP, n * 8:(n + 1) * 8].unsqueeze(2)
                            .broadcast_to((SP, 8, O)),
                        ALU.mult,
                    )
                    sp = sm_pool.tile([SP, O], f32, tag=f"spart{n}", bufs=1,
                                      name=f"sp_g{g}_{it}_{n}")
                    nc.vector.tensor_reduce(
                        sp[:],
                        tmpb[:].rearrange("p (c o) -> p o c", c=8),
                        AX.X, ALU.add,
                    )
                    sps.append(sp)
                    if n == 1:
                        nc.vector.tensor_tensor(
                            st["s01", it][:], sps[0][:], sps[1][:], ALU.add)
                    elif n == 3:
                        nc.vector.tensor_tensor(
                            st["s23", it][:], sps[2][:], sps[3][:], ALU.add)
                return f

            def squash(it, last):
                def f():
                    s_all = sm_pool.tile([SP, O], f32, tag="sall",
                                         name=f"sall_g{g}_{it}")
                    nc.vector.tensor_tensor(s_all[:], st["s01", it][:],
                                            st["s23", it][:], ALU.add)
                    # squash: v = s * sq/(1+sq)/sqrt(sq+1e-8), sq = sum s^2
                    ssq = sm_pool.tile([SP, O], f32, tag="ssq", bufs=1,
                                       name=f"ssq_g{g}_{it}")
                    sq = sm_pool.tile([SP, 1], f32, tag="sq",
                                      name=f"sq_g{g}_{it}")
                    nc.scalar.activation(ssq[:], s_all[:], AF.Square,
                                         accum_out=sq[:])
                    r1 = sm_pool.tile([SP, 1], f32, tag="r1",
                                      name=f"r1_g{g}_{it}")
                    nc.scalar.activation(r1[:], sq[:], AF.Sqrt,
                                         bias=eps_sb[:SP])
                    r2 = sm_pool.tile([SP, 1], f32, tag="r2",
                                      name=f"r2_g{g}_{it}")
                    nc.vector.scalar_tensor_tensor(
                        r2[:], sq[:], 1.0, r1[:], ALU.add, ALU.mult,
                    )
                    rr = sm_pool.tile([SP, 1], f32, tag="rr",
                                      name=f"rr_g{g}_{it}")
                    nc.vector.reciprocal(rr[:], r2[:])
                    v_all = sm_pool.tile([SP, O], f32, tag="vall",
                                         name=f"vall_g{g}_{it}")
                    nc.vector.tensor_scalar(
                        v_all[:], s_all[:], sq[:], rr[:], ALU.mult, ALU.mult,
                    )
                    st["v"] = v_all
                    if last:
                        nc.scalar.dma_start(out_d[g * SP:(g + 1) * SP, :],
                                            v_all[:])
                return f

            def psv_wbuild(it):
                def f():
                    ps_v_t = ps_t_pool.tile([O, SP], f32, tag="dt",
                                            name=f"psv_g{g}_{it}")
                    nc.tensor.transpose(ps_v_t[:], st["v"][:],
                                        identf128[:SP, :SP])
                    ps_v = ps_v_t[:]
                    for i in range(GB):
                        Wv = sm_pool.tile([P, MT, C], bf16, tag="Wv",
                                          name=f"Wv_g{g}_{it}_{i}")
                        nc.vector.tensor_tensor(
                            Wv[:O],
                            ps_v[:, i * C:(i + 1) * C:2].unsqueeze(2)
                                .broadcast_to((O, MT, C)),
                            m0[:O].rearrange("p (t c) -> p t c", t=MT),
                            ALU.mult,
                        )
                        nc.vector.tensor_tensor(
                            Wv[O:],
                            ps_v[:, i * C + 1:(i + 1) * C:2].unsqueeze(2)
                                .broadcast_to((O, MT, C)),
                            m0[O:].rearrange("p (t c) -> p t c", t=MT),
                            ALU.mult,
                        )
                        W1 = sm_pool.tile([P, MT, C], bf16, tag=f"W{i}",
                                          name=f"Wacc_g{g}_{it}_{i}")
                        nc.vector.tensor_tensor(W1[:], W_cur[i][:], Wv[:],
                                                ALU.add)
                        W_cur[i] = W1
                return f

            def delta_mm(it, i):
                def f():
                    ps_dt = ps_d_pool.tile([C, 512], f32, tag="dd",
                                           name=f"psd_g{g}_{it}_{i}")
                    st["psd", i] = ps_dt
                    for t in range(MT):
                        nc.tensor.matmul(
                            ps_dt[:],
                            W_cur[i][:, t, :],
                            UT[i, t][:],
                            start=(t == 0),
                            stop=(t == MT - 1),
                        )
                return f

            def delta_fin(it, i):
                def f():
                    if i == 0:
                        st["cnext"] = sm_pool.tile(
                            [P, GB * LT, C], bf16, tag="cij",
                            name=f"cij_g{g}_{it + 1}")
                        st["cexp"] = sm_pool.tile(
                            [P, GB * LT, C], f32, tag="cexp", bufs=1,
                            name=f"cexp_g{g}_{it + 1}")
                        st["csum"] = sm_pool.tile(
                            [P, GB * LT], f32, tag="csum",
                            name=f"csum_g{g}_{it + 1}")
                        st["crec"] = sm_pool.tile(
                            [P, GB * LT], f32, tag="crec",
                            name=f"crec_g{g}_{it + 1}")
                    ds = sm_pool.tile([C, 512], f32, tag="ds", bufs=1,
                                      name=f"ds_g{g}_{it}_{i}")
                    nc.vector.tensor_copy(ds[:], st["psd", i][:])
                    ps_t = ps_t_pool.tile([P, LT, C], f32, tag="dt",
                                          name=f"pst_g{g}_{it}_{i}")
                    for lt in range(LT):
                        nc.tensor.matmul(
                            ps_t[:, lt, :],
                            ds[:, lt * P:(lt + 1) * P],
                            identf[:],
                            is_transpose=True,
                            start=(lt == 0),
                            stop=(lt == LT - 1),
                        )
                    # ps_t IS b_ij for this batch elem; softmax for next iter
                    cexp, csum, crec = st["cexp"], st["csum"], st["crec"]
                    nc.scalar.activation(
                        cexp[:, i * LT:(i + 1) * LT, :], ps_t[:], AF.Exp)
                    nc.vector.tensor_reduce(
                        csum[:, i * LT:(i + 1) * LT],
                        cexp[:, i * LT:(i + 1) * LT, :], AX.X, ALU.add)
                    nc.vector.reciprocal(
                        crec[:, i * LT:(i + 1) * LT],
                        csum[:, i * LT:(i + 1) * LT])
                    nc.vector.tensor_tensor(
                        st["cnext"][:, i * LT:(i + 1) * LT, :],
                        cexp[:, i * LT:(i + 1) * LT, :],
                        crec[:, i * LT:(i + 1) * LT].unsqueeze(2)
                            .broadcast_to((P, LT, C)),
                        ALU.mult,
                    )
                    if i == GB - 1:
                        c_cur["t"] = st["cnext"]
                return f

            stages = [w_from_ttv]
            for i in range(GB):
                stages.append(delta_mm(0, i))
                stages.append(delta_fin(0, i))
            for it in range(1, ITERS):
                for n in range(NBANK):
                    stages.append(spass_bank(it, n))
                stages.append(squash(it, it == ITERS - 1))
                if it < ITERS - 1:
                    stages.append(psv_wbuild(it))
                    for i in range(GB):
                        stages.append(delta_mm(it, i))
                        stages.append(delta_fin(it, i))
            return stages

        INTERLEAVE = True

        def interleave(stages, ms):
            if not INTERLEAVE:
                for s in stages:
                    s()
                for f in ms:
                    f()
                return
            pi = 0
            ns = len(stages)
            for ri, s in enumerate(stages):
                s()
                tgt = min(len(ms), ((ri + 1) * len(ms)) // ns)
                while pi < tgt:
                    ms[pi]()
                    pi += 1
            while pi < len(ms):
                ms[pi]()
                pi += 1

        Us = {}
        Us[0] = make_u(0)
        dmas0, ms0 = proj_steps(0, Us[0][0], Us[0][1], {})
        for f in ms0[:MT]:
            f()
        gemv_mm()
        for f in ms0[MT:]:
            f()
        gemv_tt()
        for g in range(1, NGRP):
            Us[g] = make_u(g)
            dmas, ms = proj_steps(g, Us[g][0], Us[g][1], {})
            for d in dmas:
                d()
            interleave(routing_stages(g - 1, *Us[g - 1]), ms)
            del Us[g - 1]
        for s in routing_stages(NGRP - 1, *Us[NGRP - 1]):
            s()

    nc.compile()
    return nc


_NC_CACHE = None


def _get_nc():
    global _NC_CACHE
    if _NC_CACHE is None:
        _NC_CACHE = build_kernel()
    return _NC_CACHE


def _make_consts():
    ident = np.eye(P, dtype=_BF16)
    identf = np.eye(C, dtype=np.float32)
    identf128 = np.eye(P, dtype=np.float32)
    cc = np.arange(C)
    tt = np.arange(MT)
    m0u_row = (cc[None, :] == 2 * tt[:, None]).astype(np.float32).reshape(1, MT * C)
    m0l_row = (cc[None, :] == 2 * tt[:, None] + 1).astype(np.float32).reshape(1, MT * C)
    m0 = np.concatenate([
        np.broadcast_to(m0u_row, (O, MT * C)),
        np.broadcast_to(m0l_row, (O, MT * C)),
    ], axis=0).astype(_BF16)
    maskx = (cc[None, :] == (np.arange(P) % C)[:, None]).astype(np.float32)
    return ident, identf, identf128, m0, maskx


def kernel(inputs, fc_w, fc_b, _trace=False):
    from concourse.bass_utils import run_bass_kernel_spmd

    if _trace:
        _install_ntff_shim()

    nc = _get_nc()

    ident, identf, identf128, m0, maskx = _make_consts()
    w_bf = np.asarray(fc_w, dtype=np.float32).astype(_BF16)
    bias_f = np.asarray(fc_b, dtype=np.float32)
    bias_t = np.ascontiguousarray(bias_f.reshape(MT, P).T)
    bias16 = np.broadcast_to(16.0 * bias_f, (BPC, CO)).astype(_BF16)
    x_f = np.asarray(inputs, dtype=np.float32)
    xt_all = np.ascontiguousarray(x_f.transpose(0, 2, 1)).astype(_BF16)
    xsum_all = x_f.sum(axis=1)  # (B, D)

    in_maps = []
    for core in range(NCORES):
        xsumt = np.ascontiguousarray(
            xsum_all[core * BPC:(core + 1) * BPC].T).astype(_BF16)
        in_maps.append({
            "xt": xt_all[core * BPC:(core + 1) * BPC],
            "w": w_bf,
            "bias_t": bias_t,
            "xsumt": xsumt,
            "bias16": bias16,
            "ident": ident,
            "identf": identf,
            "identf128": identf128,
            "m0": m0,
            "mask_x": maskx,
        })

    res = run_bass_kernel_spmd(
        nc, in_maps, core_ids=list(range(NCORES)), trace=_trace,
    )
    out = np.concatenate(
        [res.results[core]["v"].reshape(BPC, C, O) for core in range(NCORES)],
        axis=0,
    )
    if _trace:
        kernel.last_exec_time_ns = res.exec_time_ns
        kernel.last_results = res
    return out


# revision 25
# speedup vs baseline: 1.9972x; 1.2105x over previous
"""Trainium2 Bass kernel for nn_CapsuleLayer (B=64, L=512, D=1024, C=32, O=64).

Strategy: data-parallel over batch across 8 NeuronCores (8 batch elements per
core), processed in 4 pipelined groups of 2 so that projection of group k+1
overlaps routing of group k on the PE. Per core, per batch element:
  u_hatT[co, l] = fc_w.T @ xT (+ fc_b)      PE, fc_w stationary
  u_hat[l, co]  = transpose(u_hatT)          PE transpose
  routing, all on-chip:
    iter 0 is algebraic: c_ij uniform => s0 = (sum_l x) @ W / 32 + 16*bias,
      computed once for all 8 batch elements with a tiny GEMV at kernel start
    s_j  = diag-extract(c_ij.T @ u_hat)      PE cross-matmul, batches packed
                                             into PSUM partitions, bank-major
                                             with a 2-bank rotation
    v_j  = squash(s_j)                       ACT/DVE
    b_ij = u_hat . (v_0+..+v_t)              PE: 16 accumulating matmuls with
                                             block-diag masked weights built
                                             from the accumulated v (W-acc)

Everything except batch sharding + input transpose/cast + x column-sums
happens on device.
"""

import contextlib
import ctypes
import sys
import types

import numpy as np
import ml_dtypes

B, L, D = 64, 512, 1024
C, O = 32, 64
CO = C * O                  # 2048
ITERS = 3
NCORES = 8
BPC = B // NCORES           # 8 batch elements per core
GB = 2                      # batch elements per routing group
NGRP = BPC // GB            # 4
P = 128
KD = D // P                 # 8 contraction chunks
MT = CO // P                # 16 m-tiles of u_hatT (= capsule pairs)
LT = L // P                 # 4 l-chunks
NBANK = CO // 512           # 4 cross-matmul column banks
SP = GB * C                 # s-cross partitions (64)

_BF16 = ml_dtypes.bfloat16

# ---------------------------------------------------------------------------
# NTFF profiling shim (used when tracing is requested by the test harness)
# ---------------------------------------------------------------------------


def _install_ntff_shim():
    if "antenv.axon_hooks" in sys.modules:
        return
    so_path = "/opt/axon/libaxon_pjrt.so"
    hook = None
    try:
        lib = ctypes.CDLL(so_path)
        if hasattr(lib, "axon_start_nrt_profile"):
            lib.axon_start_nrt_profile.argtypes = [
                ctypes.POINTER(ctypes.c_int64),
                ctypes.c_size_t,
            ]
            lib.axon_start_nrt_profile.restype = ctypes.c_int64
            lib.axon_stop_nrt_profile.argtypes = [ctypes.c_char_p]
            lib.axon_stop_nrt_profile.restype = ctypes.c_int64

            @contextlib.contextmanager
            def hook(output_dir, device_ids):
                import jax

                jax.devices()
                if device_ids:
                    ids = (ctypes.c_int64 * len(device_ids))(*device_ids)
                    rc = lib.axon_start_nrt_profile(ids, len(device_ids))
                else:
                    rc = lib.axon_start_nrt_profile(None, 0)
                if rc != 0:
                    raise RuntimeError(f"axon_start_nrt_profile rc={rc}")
                try:
                    yield
                finally:
                    n = lib.axon_stop_nrt_profile(str(output_dir).encode())
                    if n < 0:
                        raise RuntimeError(f"axon_stop_nrt_profile rc={n}")
    except OSError:
        pass
    mod = types.ModuleType("antenv.axon_hooks")
    mod.get_axon_ntff_profile_hook = lambda: hook
    mod.set_axon_ntff_profile_hook = lambda h: None
    sys.modules["antenv.axon_hooks"] = mod

    import concourse.bass_utils as bu

    bu.upload_artifacts = lambda tmpdir: tmpdir


# ---------------------------------------------------------------------------
# Kernel builder
# ---------------------------------------------------------------------------


def build_kernel():
    import concourse.bacc as bacc
    import concourse.tile as tile
    import concourse.mybir as mybir

    f32 = mybir.dt.float32
    bf16 = mybir.dt.bfloat16
    AF = mybir.ActivationFunctionType
    ALU = mybir.AluOpType
    AX = mybir.AxisListType

    nc = bacc.Bacc("TRN2", target_bir_lowering=False, debug=False)

    xt_d = nc.dram_tensor("xt", [BPC, D, L], bf16, kind="ExternalInput")
    w_d = nc.dram_tensor("w", [D, CO], bf16, kind="ExternalInput")
    bias_d = nc.dram_tensor("bias_t", [P, MT], f32, kind="ExternalInput")
    xsumt_d = nc.dram_tensor("xsumt", [D, BPC], bf16, kind="ExternalInput")
    bias16_d = nc.dram_tensor("bias16", [BPC, CO], bf16, kind="ExternalInput")
    ident_d = nc.dram_tensor("ident", [P, P], bf16, kind="ExternalInput")
    identf_d = nc.dram_tensor("identf", [C, C], f32, kind="ExternalInput")
    identf128_d = nc.dram_tensor("identf128", [P, P], f32, kind="ExternalInput")
    m0_d = nc.dram_tensor("m0", [P, MT * C], bf16, kind="ExternalInput")
    maskx_d = nc.dram_tensor("mask_x", [P, C], f32, kind="ExternalInput")
    out_d = nc.dram_tensor("v", [BPC * C, O], f32, kind="ExternalOutput")

    with tile.TileContext(nc) as tc, contextlib.ExitStack() as glb:
        const_pool = glb.enter_context(tc.tile_pool(name="consts", bufs=1))
        w_pool = glb.enter_context(tc.tile_pool(name="w", bufs=KD))
        gscr = glb.enter_context(tc.tile_pool(name="gscr", bufs=1))
        xt_pool = glb.enter_context(tc.tile_pool(name="xt", bufs=2 * KD))
        ut_pool = glb.enter_context(tc.tile_pool(name="ut", bufs=2 * GB * MT))
        u_pool = glb.enter_context(tc.tile_pool(name="u", bufs=2 * GB * LT - 1))
        sm_pool = glb.enter_context(tc.tile_pool(name="sm", bufs=2))
        pp_mm = glb.enter_context(tc.tile_pool(name="ppmm", bufs=3, space="PSUM"))
        pp_tr = glb.enter_context(tc.tile_pool(name="pptr", bufs=1, space="PSUM"))
        ps_s_pool = glb.enter_context(tc.tile_pool(name="pss", bufs=2, space="PSUM"))
        ps_d_pool = glb.enter_context(tc.tile_pool(name="psd", bufs=1, space="PSUM"))
        ps_t_pool = glb.enter_context(tc.tile_pool(name="pst", bufs=1, space="PSUM"))

        # --- constants; g0 xt DMAs issued early so the PE can start ---
        w_sb = []
        for k in range(KD):
            wt = w_pool.tile([P, CO], bf16, tag="w", name=f"w{k}")
            nc.sync.dma_start(wt[:], w_d[k * P:(k + 1) * P, :])
            w_sb.append(wt)
        xt_pre = {}
        for i in range(GB):
            for k in range(KD):
                t = xt_pool.tile([P, L], bf16, tag="xt", name=f"xt_g0_{i}_{k}")
                nc.sync.dma_start(t[:], xt_d[i, k * P:(k + 1) * P, :])
                xt_pre[i, k] = t
        ident = const_pool.tile([P, P], bf16, name="ident")
        nc.sync.dma_start(ident[:], ident_d[:])
        bias_sb = const_pool.tile([P, MT], f32, name="bias_sb")
        nc.sync.dma_start(bias_sb[:], bias_d[:])
        identf = const_pool.tile([C, C], f32, name="identf")
        nc.scalar.dma_start(identf[:], identf_d[:])
        identf128 = const_pool.tile([P, P], f32, name="identf128")
        nc.scalar.dma_start(identf128[:], identf128_d[:])
        m0 = const_pool.tile([P, MT * C], bf16, name="m0")
        nc.scalar.dma_start(m0[:], m0_d[:])
        maskx = const_pool.tile([P, C], f32, name="maskx")
        nc.scalar.dma_start(maskx[:], maskx_d[:])
        eps_sb = const_pool.tile([P, 1], f32, name="eps_sb")
        nc.vector.memset(eps_sb[:], 1e-8)
        # ttv[(j,o), m, b]: partition j*64+o holds v0[b, c=2m+j, o]
        ttv = const_pool.tile([P, MT, BPC], bf16, name="ttv")

        # --- iter-0 GEMV: s0 = xsum @ W / 32 + 16*bias for all 8 elems.
        # Issued into the PE queue between the two g0 projections; the tiny
        # squash chain runs on DVE/ACT under the second projection.
        xs_sb = gscr.tile([P, KD, BPC], bf16, name="xs_sb")
        nc.sync.dma_start(
            xs_sb[:], xsumt_d[:].rearrange("(k p) b -> p k b", p=P))
        bias16_sb = gscr.tile([BPC, CO], bf16, tag="g8a", name="bias16")
        nc.sync.dma_start(bias16_sb[:], bias16_d[:])
        v0_sb = gscr.tile([BPC, CO], bf16, name="v0_sb")

        def gemv_mm():
            s0_sb = gscr.tile([BPC, CO], bf16, name="s0_sb")
            for n in range(NBANK):
                ps0 = pp_mm.tile([BPC, 512], f32, tag="mm", name=f"ps0_{n}")
                for k in range(KD):
                    nc.tensor.matmul(
                        ps0[:], xs_sb[:, k, :],
                        w_sb[k][:, n * 512:(n + 1) * 512],
                        start=(k == 0), stop=(k == KD - 1),
                    )
                nc.vector.scalar_tensor_tensor(
                    s0_sb[:, n * 512:(n + 1) * 512], ps0[:], 1.0 / 32.0,
                    bias16_sb[:, n * 512:(n + 1) * 512], ALU.mult, ALU.add,
                )
            # squash s0 -> v0  (scale varies along free dim (c))
            tmp0 = gscr.tile([BPC, CO], bf16, tag="g8a", name="tmp0")
            nc.vector.tensor_tensor(tmp0[:], s0_sb[:], s0_sb[:], ALU.mult)
            sq0 = gscr.tile([BPC, C], f32, name="sq0")
            nc.vector.tensor_reduce(
                sq0[:], tmp0[:].rearrange("b (c o) -> b c o", c=C),
                AX.X, ALU.add)
            r10 = gscr.tile([BPC, C], f32, name="r10")
            nc.scalar.activation(r10[:], sq0[:], AF.Sqrt, bias=eps_sb[:BPC])
            r20 = gscr.tile([BPC, C], f32, name="r20")
            nc.vector.scalar_tensor_tensor(
                r20[:], sq0[:], 1.0, r10[:], ALU.add, ALU.mult)
            rr0 = gscr.tile([BPC, C], f32, name="rr0")
            nc.vector.reciprocal(rr0[:], r20[:])
            sc0 = gscr.tile([BPC, C], f32, name="sc0")
            nc.vector.tensor_tensor(sc0[:], sq0[:], rr0[:], ALU.mult)
            nc.vector.tensor_tensor(
                v0_sb[:].rearrange("b (c o) -> b c o", c=C),
                s0_sb[:].rearrange("b (c o) -> b c o", c=C),
                sc0[:].unsqueeze(2).broadcast_to((BPC, C, O)),
                ALU.mult,
            )

        def gemv_tt():
            tt_ps = ps_t_pool.tile([P, MT * BPC], bf16, tag="dt",
                                   name="tt_ps")
            for m in range(MT):
                nc.tensor.matmul(
                    tt_ps[:, m * BPC:(m + 1) * BPC],
                    v0_sb[:, m * P:(m + 1) * P],
                    ident[:BPC, :BPC],
                    is_transpose=True,
                    start=(m == 0), stop=(m == MT - 1),
                )
            nc.vector.tensor_copy(
                ttv[:], tt_ps[:].rearrange("p (m b) -> p m b", m=MT))

        # ------------------------------------------------------------------
        # Software-pipelined schedule: the PE queue interleaves projection
        # m-tiles of group g with the routing stages of group g-1 so that
        # routing's serial DVE/ACT chains never leave the PE idle.
        # ------------------------------------------------------------------

        def make_u(g):
            U = {}
            for i in range(GB):
                for lt in range(LT):
                    U[i, lt] = u_pool.tile([P, CO], bf16, tag="u",
                                           name=f"u_g{g}_{i}_{lt}")
            return U, {}

        def proj_steps(g, U, UT, xt_sb):
            def dma_step(i, b):
                def f():
                    for k in range(KD):
                        t = xt_pool.tile([P, L], bf16, tag="xt",
                                         name=f"xt_g{g}_{i}_{k}")
                        nc.sync.dma_start(t[:], xt_d[b, k * P:(k + 1) * P, :])
                        xt_sb[i, k] = t
                return f

            pcell = [None]

            def mm_step(i, m):
                def f():
                    ps = pp_mm.tile([P, 512], f32, tag="mm",
                                    name=f"ps_g{g}_{m}_{i}")
                    for k in range(KD):
                        nc.tensor.matmul(
                            ps[:],
                            w_sb[k][:, m * P:(m + 1) * P],
                            xt_sb[i, k][:],
                            start=(k == 0),
                            stop=(k == KD - 1),
                        )
                    ut = ut_pool.tile([P, L], bf16, tag="ut",
                                      name=f"ut_g{g}_{i}_{m}")
                    nc.scalar.activation(
                        ut[:], ps[:], AF.Identity,
                        bias=bias_sb[:, m:m + 1],
                    )
                    UT[i, m] = ut
                return f

            def tr_step(i, p):
                # transpose the (2p, 2p+1) m-pair; issued one m-step late so
                # the ACT bias-add is off the PE critical path
                def f():
                    ptr = pp_tr.tile([P, LT, 2, P], bf16, tag="tr",
                                     name=f"ptr_g{g}_{p}_{i}")
                    for mh in range(2):
                        ut = UT[i, 2 * p + mh]
                        for lt in range(LT):
                            nc.tensor.matmul(
                                ptr[:, lt, mh, :],
                                ut[:, lt * P:(lt + 1) * P],
                                ident[:],
                                is_transpose=True,
                                start=(mh == 0 and lt == 0),
                                stop=(mh == 1 and lt == LT - 1),
                            )
                    for lt in range(LT):
                        nc.vector.tensor_copy(
                            U[i, lt][:, 2 * p * P:(2 * p + 2) * P],
                            ptr[:, lt, :, :],
                        )
                return f

            dmas, ms = [], []
            for i in range(GB):
                b = g * GB + i
                if g == 0:
                    for k in range(KD):
                        xt_sb[i, k] = xt_pre[i, k]
                else:
                    dmas.append(dma_step(i, b))
                for m in range(MT):
                    ms.append(mm_step(i, m))
                    if m >= 2 and m % 2 == 0:
                        ms.append(tr_step(i, (m - 2) // 2))
                ms.append(tr_step(i, MT // 2 - 2))
                ms.append(tr_step(i, MT // 2 - 1))
            return dmas, ms

        def routing_stages(g, U, UT):
            bs = [g * GB + i for i in range(GB)]
            st = {}
            W_cur = {}
            c_cur = {}

            def c_get(i, lt):
                return c_cur["t"][:, i * LT + lt, :]

            def w_from_ttv():
                for i in range(GB):
                    W0 = sm_pool.tile([P, MT, C], bf16, tag=f"W{i}",
                                      name=f"W_g{g}_0_{i}")
                    nc.vector.tensor_tensor(
                        W0[:O],
                        ttv[:O, :, bs[i]].unsqueeze(2)
                            .broadcast_to((O, MT, C)),
                        m0[:O].rearrange("p (t c) -> p t c", t=MT),
                        ALU.mult,
                    )
                    nc.vector.tensor_tensor(
                        W0[O:],
                        ttv[O:, :, bs[i]].unsqueeze(2)
                            .broadcast_to((O, MT, C)),
                        m0[O:].rearrange("p (t c) -> p t c", t=MT),
                        ALU.mult,
                    )
                    W_cur[i] = W0

            def spass_bank(it, n):
                def f():
                    if n == 0:
                        st["sps", it] = []
                        st["s01", it] = sm_pool.tile([SP, O], f32, tag="s01",
                                                     name=f"s01_g{g}_{it}")
                        st["s23", it] = sm_pool.tile([SP, O], f32, tag="s23",
                                                     name=f"s23_g{g}_{it}")
                    sps = st["sps", it]
                    ps_s = ps_s_pool.tile([SP, 512], f32, tag="ss",
                                          name=f"pss_g{g}_{it}_{n}")
                    for lt in range(LT):
                        for i in range(GB):
                            nc.tensor.matmul(
                                ps_s[i * C:(i + 1) * C, :],
                                c_get(i, lt),
                                U[i, lt][:, n * 512:(n + 1) * 512],
                                start=(lt == 0),
                                stop=(lt == LT - 1),
                                tile_position=(0, i * C),
                                skip_group_check=(i > 0),
                            )
                    tmpb = sm_pool.tile([SP, 512], f32, tag="tmpb",
                                        name=f"tmpb_g{g}_{it}_{n}")
                    nc.vector.tensor_tensor(
                        tmpb[:].rearrange("p (c o) -> p c o", c=8),
                        ps_s[:].rearrange("p (c o) -> p c o", c=8),
                        maskx[:SP, n * 8:(n + 1) * 8].unsqueeze(2)
                            .broadcast_to((SP, 8, O)),
                        ALU.mult,
                    )
                    sp = sm_pool.tile([SP, O], f32, tag=f"spart{n}", bufs=1,
                                      name=f"sp_g{g}_{it}_{n}")
                    nc.vector.tensor_reduce(
                        sp[:],
                        tmpb[:].rearrange("p (c o) -> p o c", c=8),
                        AX.X, ALU.add,
                    )
                    sps.append(sp)
                    if n == 1:
                        nc.vector.tensor_tensor(
                            st["s01", it][:], sps[0][:], sps[1][:], ALU.add)
                    elif n == 3:
                        nc.vector.tensor_tensor(
                            st["s23", it][:], sps[2][:], sps[3][:], ALU.add)
                return f

            def squash(it, last):
                def f():
                    s_all = sm_pool.tile([SP, O], f32, tag="sall",
                                         name=f"sall_g{g}_{it}")
                    nc.vector.tensor_tensor(s_all[:], st["s01", it][:],
                                            st["s23", it][:], ALU.add)
                    # squash: v = s * sq/(1+sq)/sqrt(sq+1e-8), sq = sum s^2
                    ssq = sm_pool.tile([SP, O], f32, tag="ssq", bufs=1,
                                       name=f"ssq_g{g}_{it}")
                    sq = sm_pool.tile([SP, 1], f32, tag="sq",
                                      name=f"sq_g{g}_{it}")
                    nc.scalar.activation(ssq[:], s_all[:], AF.Square,
                                         accum_out=sq[:])
                    r1 = sm_pool.tile([SP, 1], f32, tag="r1",
                                      name=f"r1_g{g}_{it}")
                    nc.scalar.activation(r1[:], sq[:], AF.Sqrt,
                                         bias=eps_sb[:SP])
                    r2 = sm_pool.tile([SP, 1], f32, tag="r2",
                                      name=f"r2_g{g}_{it}")
                    nc.vector.scalar_tensor_tensor(
                        r2[:], sq[:], 1.0, r1[:], ALU.add, ALU.mult,
                    )
                    rr = sm_pool.tile([SP, 1], f32, tag="rr",
                                      name=f"rr_g{g}_{it}")
                    nc.vector.reciprocal(rr[:], r2[:])
                    v_all = sm_pool.tile([SP, O], f32, tag="vall",
                                         name=f"vall_g{g}_{it}")
                    nc.vector.tensor_scalar(
                        v_all[:], s_all[:], sq[:], rr[:], ALU.mult, ALU.mult,
                    )
                    st["v"] = v_all
                    if last:
                        nc.scalar.dma_start(out_d[g * SP:(g + 1) * SP, :],
                                            v_all[:])
                return f

            def psv_wbuild(it):
                def f():
                    ps_v_t = ps_t_pool.tile([O, SP], f32, tag="dt",
                                            name=f"psv_g{g}_{it}")
                    nc.tensor.transpose(ps_v_t[:], st["v"][:],
                                        identf128[:SP, :SP])
                    ps_v = ps_v_t[:]
                    for i in range(GB):
                        Wv = sm_pool.tile([P, MT, C], bf16, tag="Wv",
                                          name=f"Wv_g{g}_{it}_{i}")
                        nc.vector.tensor_tensor(
                            Wv[:O],
                            ps_v[:, i * C:(i + 1) * C:2].unsqueeze(2)
                                .broadcast_to((O, MT, C)),
                            m0[:O].rearrange("p (t c) -> p t c", t=MT),
                            ALU.mult,
                        )
                        nc.vector.tensor_tensor(
                            Wv[O:],
                            ps_v[:, i * C + 1:(i + 1) * C:2].unsqueeze(2)
                                .broadcast_to((O, MT, C)),
                            m0[O:].rearrange("p (t c) -> p t c", t=MT),
                            ALU.mult,
                        )
                        W1 = sm_pool.tile([P, MT, C], bf16, tag=f"W{i}",
                                          name=f"Wacc_g{g}_{it}_{i}")
                        nc.vector.tensor_tensor(W1[:], W_cur[i][:], Wv[:],
                                                ALU.add)
                        W_cur[i] = W1
                return f

            def delta_mm(it, i):
                def f():
                    ps_dt = ps_d_pool.tile([C, 512], f32, tag="dd",
                                           name=f"psd_g{g}_{it}_{i}")
                    st["psd", i] = ps_dt
                    for t in range(MT):
                        nc.tensor.matmul(
                            ps_dt[:],
                            W_cur[i][:, t, :],
                            UT[i, t][:],
                            start=(t == 0),
                            stop=(t == MT - 1),
                        )
                return f

            def delta_fin(it, i):
                def f():
                    if i == 0:
                        st["cnext"] = sm_pool.tile(
                            [P, GB * LT, C], bf16, tag="cij",
                            name=f"cij_g{g}_{it + 1}")
                        st["cexp"] = sm_pool.tile(
                            [P, GB * LT, C], f32, tag="cexp", bufs=1,
                            name=f"cexp_g{g}_{it + 1}")
                        st["csum"] = sm_pool.tile(
                            [P, GB * LT], f32, tag="csum",
                            name=f"csum_g{g}_{it + 1}")
                        st["crec"] = sm_pool.tile(
                            [P, GB * LT], f32, tag="crec",
                            name=f"crec_g{g}_{it + 1}")
                    ds = sm_pool.tile([C, 512], f32, tag="ds", bufs=1,
                                      name=f"ds_g{g}_{it}_{i}")
                    nc.vector.tensor_copy(ds[:], st["psd", i][:])
                    ps_t = ps_t_pool.tile([P, LT, C], f32, tag="dt",
                                          name=f"pst_g{g}_{it}_{i}")
                    for lt in range(LT):
                        nc.tensor.matmul(
                            ps_t[:, lt, :],
                            ds[:, lt * P:(lt + 1) * P],
                            identf[:],
                            is_transpose=True,
                            start=(lt == 0),
                            stop=(lt == LT - 1),
                        )
                    # ps_t IS b_ij for this batch elem; softmax for next iter
                    cexp, csum, crec = st["cexp"], st["csum"], st["crec"]
                    nc.scalar.activation(
                        cexp[:, i * LT:(i + 1) * LT, :], ps_t[:], AF.Exp)
                    nc.vector.tensor_reduce(
                        csum[:, i * LT:(i + 1) * LT],
                        cexp[:, i * LT:(i + 1) * LT, :], AX.X, ALU.add)
                    nc.vector.reciprocal(
                        crec[:, i * LT:(i + 1) * LT],
                        csum[:, i * LT:(i + 1) * LT])
                    nc.vector.tensor_tensor(
                        st["cnext"][:, i * LT:(i + 1) * LT, :],
                        cexp[:, i * LT:(i + 1) * LT, :],
                        crec[:, i * LT:(i + 1) * LT].unsqueeze(2)
                            .broadcast_to((P, LT, C)),
                        ALU.mult,
                    )
                    if i == GB - 1:
                        c_cur["t"] = st["cnext"]
                return f

            stages = [w_from_ttv]
            for i in range(GB):
                stages.append(delta_mm(0, i))
                stages.append(delta_fin(0, i))
            for it in range(1, ITERS):
                for n in range(NBANK):
                    stages.append(spass_bank(it, n))
                stages.append(squash(it, it == ITERS - 1))
                if it < ITERS - 1:
                    stages.append(psv_wbuild(it))
                    for i in range(GB):
                        stages.append(delta_mm(it, i))
                        stages.append(delta_fin(it, i))
            return stages

        INTERLEAVE = True

        def interleave(stages, ms):
            if not INTERLEAVE:
                for s in stages:
                    s()
                for f in ms:
                    f()
                return
            pi = 0
            ns = len(stages)
            for ri, s in enumerate(stages):
                s()
                tgt = min(len(ms), ((ri + 1) * len(ms)) // ns)
                while pi < tgt:
                    ms[pi]()
                    pi += 1
            while pi < len(ms):
                ms[pi]()
                pi += 1

        Us = {}
        Us[0] = make_u(0)
        dmas0, ms0 = proj_steps(0, Us[0][0], Us[0][1], {})
        for f in ms0[:MT]:
            f()
        gemv_mm()
        for f in ms0[MT:]:
            f()
        gemv_tt()
        for g in range(1, NGRP):
            Us[g] = make_u(g)
            dmas, ms = proj_steps(g, Us[g][0], Us[g][1], {})
            for d in dmas:
                d()
            interleave(routing_stages(g - 1, *Us[g - 1]), ms)
            del Us[g - 1]
        for s in routing_stages(NGRP - 1, *Us[NGRP - 1]):
            s()

    nc.compile()
    return nc


_NC_CACHE = None


def _get_nc():
    global _NC_CACHE
    if _NC_CACHE is None:
        _NC_CACHE = build_kernel()
    return _NC_CACHE


def _make_consts():
    ident = np.eye(P, dtype=_BF16)
    identf = np.eye(C, dtype=np.float32)
    identf128 = np.eye(P, dtype=np.float32)
    cc = np.arange(C)
    tt = np.arange(MT)
    m0u_row = (cc[None, :] == 2 * tt[:, None]).astype(np.float32).reshape(1, MT * C)
    m0l_row = (cc[None, :] == 2 * tt[:, None] + 1).astype(np.float32).reshape(1, MT * C)
    m0 = np.concatenate([
        np.broadcast_to(m0u_row, (O, MT * C)),
        np.broadcast_to(m0l_row, (O, MT * C)),
    ], axis=0).astype(_BF16)
    maskx = (cc[None, :] == (np.arange(P) % C)[:, None]).astype(np.float32)
    return ident, identf, identf128, m0, maskx


def kernel(inputs, fc_w, fc_b, _trace=False):
    from concourse.bass_utils import run_bass_kernel_spmd

    if _trace:
        _install_ntff_shim()

    nc = _get_nc()

    ident, identf, identf128, m0, maskx = _make_consts()
    w_bf = np.asarray(fc_w, dtype=np.float32).astype(_BF16)
    bias_f = np.asarray(fc_b, dtype=np.float32)
    bias_t = np.ascontiguousarray(bias_f.reshape(MT, P).T)
    bias16 = np.broadcast_to(16.0 * bias_f, (BPC, CO)).astype(_BF16)
    x_f = np.asarray(inputs, dtype=np.float32)
    xt_all = np.ascontiguousarray(x_f.transpose(0, 2, 1)).astype(_BF16)
    xsum_all = x_f.sum(axis=1)  # (B, D)

    in_maps = []
    for core in range(NCORES):
        xsumt = np.ascontiguousarray(
            xsum_all[core * BPC:(core + 1) * BPC].T).astype(_BF16)
        in_maps.append({
            "xt": xt_all[core * BPC:(core + 1) * BPC],
            "w": w_bf,
            "bias_t": bias_t,
            "xsumt": xsumt,
            "bias16": bias16,
            "ident": ident,
            "identf": identf,
            "identf128": identf128,
            "m0": m0,
            "mask_x": maskx,
        })

    res = run_bass_kernel_spmd(
        nc, in_maps, core_ids=list(range(NCORES)), trace=_trace,
    )
    out = np.concatenate(
        [res.results[core]["v"].reshape(BPC, C, O) for core in range(NCORES)],
        axis=0,
    )
    if _trace:
        kernel.last_exec_time_ns = res.exec_time_ns
        kernel.last_results = res
    return out
